# revision 11
# baseline (speedup 1.0000x reference)
"""DiT attention block (QKV proj + QK-RMSNorm + RoPE + softmax attention + out proj)
as a Bass/Tile kernel for 8 Trainium2 NeuronCores.

Sharding (zero cross-core communication):
  core c -> batch b = c//2, sequence half = c%2.
  Each core computes output rows [half*1024, half*1024+1024) of batch b:
    - K, V are computed for the full 2048-row sequence of batch b (duplicated
      within a core pair); Q only for the core's own 1024 rows.
    - attention + out-proj for the core's 1024 query rows.
  Host concatenates the 8 disjoint row blocks into the full [4, 2048, 1024].

Layout strategy per core:
  - x chunks are transposed on the tensor engine (via identity matmul) to get
    the contraction dim (D) on partitions for the QKV projections.
  - Q/K are projected in "natural" [l, d] layout, RMS-normed + roped there
    (free-dim reduces), then transposed per head to q^T/k^T [hd, l], packed
    two heads per 128 partitions.
  - S^T = k^T.T @ q^T is computed per (head, 128-row lk chunk) into PSUM
    [128 lk, 1024 lq]; ScalarE applies exp(0.125*S) into SBUF; the AV matmul
    uses [v_h | ones] as the stationary operand so PSUM row 64 accumulates the
    softmax denominator for free.  attn^T is normalized afterwards and used
    directly as the stationary operand of the out projection.
  - Matmuls run in float32r (fp32 bits, FP22 multiplies) which streams at
    bf16 rate for moving free dims >= 256.
"""

import sys

if "/opt/trn_rl_repo" not in sys.path:
    sys.path.insert(0, "/opt/trn_rl_repo")

from contextlib import ExitStack

import numpy as np

import concourse.bass as bass
import concourse.tile as tile
from concourse import mybir, bass_utils
from concourse.masks import make_identity
from concourse.vector_clock import ScopedClock, VectorClock

B, L, D, H = 4, 2048, 1024, 16
HD = D // H          # 64
HHD = HD // 2        # 32
EPS = 1e-6
THETA = 10000.0
N_CORES = 8
LQ = L // 2          # query rows per core
P = 128
NCK = L // P         # 16 l-chunks for K/V
NCQ = LQ // P        # 8 l-chunks for Q
NDC = D // P         # 8 contraction chunks
F32 = mybir.dt.float32
FR = mybir.dt.float32r
AF = mybir.ActivationFunctionType


def _patch_tile_drain():
    """This container's walrus rejects >1 sem wait per instruction.
    Tile's kernel-tail drain waits on every active proc at once; split those
    waits across single-wait NOPs on SP so the drain itself needs none."""
    if getattr(tile.TileContext, "_drain_split_patched", False):
        return

    def _patched(self, tick_clock, wait_clock):
        vc = tick_clock.global_clock
        n = len(vc)
        cur = VectorClock([0] * n)
        for proc in range(n):
            t = vc[proc]
            if t > 0:
                nop = self.nc.sync.nop(hint=f"drainsplit_{proc}", nofuse=True)
                req = VectorClock([0] * n)
                req.require_at_least(proc, t)
                wait_clock.add_sem_waits(
                    nop.ins, ScopedClock({None: req}), ScopedClock({None: cur.copy()})
                )
                cur.require_at_least(proc, t)
        drain_inst = self.nc.sync.drain()
        wait_clock.add_sem_waits(
            drain_inst.ins, ScopedClock({None: vc}), ScopedClock({None: cur})
        )
        self.nc.all_engine_barrier()
        popped = self.nc._tile_sem_poison_stack.pop()
        assert popped is self._sem_poison
        self.nc.clear_and_free_semaphores(list(self.sems.allocated().values()))
        self.nc.all_engine_barrier()

    tile.TileContext._drain_and_barrier = _patched
    tile.TileContext._drain_split_patched = True


def _split_waits(nc, maxw=1):
    """This walrus build allows at most one sync wait per instruction.
    After Tile has assigned semaphores, hoist excess waits onto NOPs
    inserted just before the over-subscribed instruction (same engine,
    same block) — semantically identical, since all waits must clear
    before the instruction executes either way."""
    nid = 0
    for fn in nc.m.functions:
        for bb in fn.blocks:
            insts = list(bb.instructions)
            new = []
            changed = False
            for inst in insts:
                si = inst.sync_info
                if si is not None and si.on_wait is not None and len(si.on_wait) > maxw:
                    waits = list(si.on_wait)
                    extra, keep = waits[:-maxw], waits[-maxw:]
                    for i in range(0, len(extra), maxw):
                        nid += 1
                        new.append(mybir.InstNoOp(
                            name=f"I-wsplit-{nid}", engine=inst.engine,
                            sync_info=mybir.SyncInfo(
                                on_wait=extra[i : i + maxw], on_update=[]),
                        ))
                    inst.sync_info = mybir.SyncInfo(
                        on_wait=keep, on_update=list(si.on_update))
                    changed = True
                new.append(inst)
            if changed:
                bb.instructions = new


def _bcast_free(ap, repeat, at):
    """Insert a step-0 free dim of size `repeat` at free-dim position `at`
    (0 = right after the partition dim)."""
    new = ap.copy()
    new.ap = new.ap[: 1 + at] + [[0, repeat]] + new.ap[1 + at :]
    return new


def _fr(ap):
    if ap.dtype == FR:
        return ap
    return ap.bitcast(FR)


def _build_program(use_bq, use_bk, use_bv, use_bout, use_qnw, use_knw):
    nc = bass.Bass("TRN2", target_bir_lowering=False, debug=False,
                   num_devices=N_CORES)

    xb = nc.dram_tensor("xb", [L, D], F32, kind="ExternalInput").ap()
    xq = nc.dram_tensor("xq", [LQ, D], F32, kind="ExternalInput").ap()
    wq = nc.dram_tensor("wq", [D, D], F32, kind="ExternalInput").ap()
    wk = nc.dram_tensor("wk", [D, D], F32, kind="ExternalInput").ap()
    wv = nc.dram_tensor("wv", [D, D], F32, kind="ExternalInput").ap()
    wout = nc.dram_tensor("wout", [D, D], F32, kind="ExternalInput").ap()
    cosk = nc.dram_tensor("cosk", [L, HHD], F32, kind="ExternalInput").ap()
    sink = nc.dram_tensor("sink", [L, HHD], F32, kind="ExternalInput").ap()
    cosq = nc.dram_tensor("cosq", [LQ, HHD], F32, kind="ExternalInput").ap()
    sinq = nc.dram_tensor("sinq", [LQ, HHD], F32, kind="ExternalInput").ap()
    bq = bk = bv = bo = qnw = knw = None
    if use_bq:
        bq = nc.dram_tensor("bq", [1, D], F32, kind="ExternalInput").ap()
    if use_bk:
        bk = nc.dram_tensor("bk", [1, D], F32, kind="ExternalInput").ap()
    if use_bv:
        bv = nc.dram_tensor("bv", [1, D], F32, kind="ExternalInput").ap()
    if use_bout:
        bo = nc.dram_tensor("bout", [1, D], F32, kind="ExternalInput").ap()
    if use_qnw:
        qnw = nc.dram_tensor("qnw", [1, HD], F32, kind="ExternalInput").ap()
    if use_knw:
        knw = nc.dram_tensor("knw", [1, HD], F32, kind="ExternalInput").ap()
    out = nc.dram_tensor("out", [LQ, D], F32, kind="ExternalOutput").ap()

    with tile.TileContext(nc) as tc, ExitStack() as ctx:
        pers = ctx.enter_context(tc.tile_pool(name="pers", bufs=1))
        dpool = ctx.enter_context(tc.tile_pool(name="dram", bufs=1, space="DRAM"))

        ident = pers.tile([P, P], F32, tag="ident")
        make_identity(nc, ident)

        cosk_sb = pers.tile([P, NCK, HHD], F32, tag="cosk")
        sink_sb = pers.tile([P, NCK, HHD], F32, tag="sink")
        cosq_sb = pers.tile([P, NCQ, HHD], F32, tag="cosq")
        sinq_sb = pers.tile([P, NCQ, HHD], F32, tag="sinq")
        nc.sync.dma_start(out=cosk_sb, in_=cosk.rearrange("(c p) f -> p c f", p=P))
        nc.sync.dma_start(out=sink_sb, in_=sink.rearrange("(c p) f -> p c f", p=P))
        nc.sync.dma_start(out=cosq_sb, in_=cosq.rearrange("(c p) f -> p c f", p=P))
        nc.sync.dma_start(out=sinq_sb, in_=sinq.rearrange("(c p) f -> p c f", p=P))

        # packed transposed activations: pair tile p holds head 2p in
        # partitions 0:64 and head 2p+1 in partitions 64:128.
        kT = pers.tile([P, H // 2, L], FR, tag="kT")
        qT = pers.tile([P, H // 2, LQ], FR, tag="qT")

        ones16 = pers.tile([P, H], F32, tag="ones16")
        nc.vector.memset(ones16, 1.0)
        eps_sb = pers.tile([P, 1], F32, tag="eps")
        nc.vector.memset(eps_sb, EPS)
        ones1 = None
        if use_bq or use_bk or use_bv:
            ones1f = pers.tile([1, P], F32, tag="ones1f")
            nc.vector.memset(ones1f, 1.0)
            ones1 = pers.tile([1, P], FR, tag="ones1")
            nc.vector.tensor_copy(ones1, ones1f)
        qnw_b = knw_b = bout_b = None
        if use_qnw:
            qnw_b = pers.tile([P, HD], F32, tag="qnw_b")
            nc.sync.dma_start(
                out=qnw_b,
                in_=bass.AP(tensor=qnw.tensor, offset=qnw.offset,
                            ap=[[0, P], [1, HD]]),
            )
        if use_knw:
            knw_b = pers.tile([P, HD], F32, tag="knw_b")
            nc.sync.dma_start(
                out=knw_b,
                in_=bass.AP(tensor=knw.tensor, offset=knw.offset,
                            ap=[[0, P], [1, HD]]),
            )
        if use_bout:
            bout_b = pers.tile([P, D], F32, tag="bout_b")
            nc.sync.dma_start(
                out=bout_b,
                in_=bass.AP(tensor=bo.tensor, offset=bo.offset,
                            ap=[[0, P], [1, D]]),
            )

        # DRAM staging: v with a ones column appended (AV stationary operand),
        # and per-head softmax denominators for the broadcast bounce.
        vstage = dpool.tile([H, L, HD + 1], F32, tag="vstage")
        invstage = dpool.tile([H, LQ], F32, tag="invstage")
        for h in range(H):
            nc.sync.dma_start(out=vstage[h, :, HD : HD + 1], in_=ones16)

        def load_w(pool, w_dram, tag):
            w_sb = pool.tile([P, NDC, D], FR, tag=tag)
            nc.sync.dma_start(
                out=w_sb,
                in_=w_dram.rearrange("(j p) n -> p j n", p=P).bitcast(FR),
            )
            return w_sb

        def load_bias(pool, b_dram, tag):
            b_sb = pool.tile([1, D], FR, tag=tag)
            nc.sync.dma_start(out=b_sb, in_=b_dram.bitcast(FR))
            return b_sb

        def proj_chunk(x_dram, ci, w_sb, b_sb, xpool, tppool, pspool):
            """Project one 128-row chunk: returns PSUM [128, D] = x_chunk @ W (+b)."""
            xc = xpool.tile([P, D], F32, tag="xc")
            nc.sync.dma_start(out=xc, in_=x_dram[ci * P : (ci + 1) * P, :])
            tp = tppool.tile([P, NDC * P], F32, tag="tp")
            for j in range(NDC):
                nc.tensor.transpose(
                    tp[:, j * P : (j + 1) * P], xc[:, j * P : (j + 1) * P],
                    ident,
                )
            xt = xpool.tile([P, NDC, P], FR, tag="xt")
            nc.scalar.copy(xt, tp.rearrange("p (j q) -> p j q", j=NDC))
            ps = pspool.tile([P, D], F32, tag="ps")
            for n0 in range(0, D, 512):
                for j in range(NDC):
                    nc.tensor.matmul(
                        ps[:, n0 : n0 + 512],
                        _fr(xt[:, j, :]),
                        _fr(w_sb[:, j, n0 : n0 + 512]),
                        start=(j == 0),
                        stop=(j == NDC - 1 and b_sb is None),
                    )
                if b_sb is not None:
                    nc.tensor.matmul(
                        ps[:, n0 : n0 + 512],
                        _fr(ones1),
                        _fr(b_sb[:, n0 : n0 + 512]),
                        start=False,
                        stop=True,
                    )
            return ps

        def norm_rope(ps, cos_ap, sin_ap, nw_b, stg):
            """RMSNorm over head_dim + rotary embed, from PSUM [128, D] in
            natural layout; returns SBUF tile [128, H, HD]."""
            sq = stg.tile([P, D], F32, tag="sq")
            nc.scalar.activation(sq, ps, AF.Square)
            ss = stg.tile([P, H], F32, tag="ss")
            nc.vector.tensor_reduce(
                ss, sq.rearrange("p (h d) -> p h d", h=H),
                axis=mybir.AxisListType.X, op=mybir.AluOpType.add,
            )
            inv = stg.tile([P, H], F32, tag="inv")
            nc.scalar.activation(inv, ss, AF.Sqrt, scale=1.0 / HD, bias=eps_sb)
            nc.vector.reciprocal(inv, inv)
            ps3 = ps.rearrange("p (h d) -> p h d", h=H)
            kn = stg.tile([P, H, HD], F32, tag="kn")
            nc.vector.tensor_mul(kn, ps3, _bcast_free(inv, HD, 1))
            if nw_b is not None:
                nc.vector.tensor_mul(kn, kn, _bcast_free(nw_b, H, 0))
            t1 = kn[:, :, 0:HHD]
            t2 = kn[:, :, HHD:HD]
            cosc = _bcast_free(cos_ap, H, 0)
            sinc = _bcast_free(sin_ap, H, 0)
            ra = stg.tile([P, H, HHD], F32, tag="ra")
            rb = stg.tile([P, H, HHD], F32, tag="rb")
            rot = stg.tile([P, H, HD], F32, tag="rot")
            nc.vector.tensor_mul(ra, t1, cosc)
            nc.vector.tensor_mul(rb, t2, sinc)
            nc.vector.tensor_sub(rot[:, :, 0:HHD], ra, rb)
            nc.vector.tensor_mul(ra, t1, sinc)
            nc.vector.tensor_mul(rb, t2, cosc)
            nc.vector.tensor_add(rot[:, :, HHD:HD], ra, rb)
            return rot

        def transpose_heads(rot, dstT, ci, tp2pool):
            """Per-head PE transpose of [128, HD] -> [HD, 128], packed into
            dstT[(h%2)*64:(h%2)*64+64, h//2, ci*128:ci*128+128]."""
            for g in range(2):
                t2 = tp2pool.tile([HD, 8, P], F32, tag="t2")
                for hh in range(8):
                    h = g * 8 + hh
                    nc.tensor.transpose(
                        t2[:, hh, :], rot[:, h, :], ident
                    )
                for hh in range(8):
                    h = g * 8 + hh
                    dst = dstT[(h % 2) * HD : (h % 2 + 1) * HD, h // 2,
                               ci * P : (ci + 1) * P]
                    if hh % 2 == 0:
                        nc.vector.tensor_copy(dst, t2[:, hh, :])
                    else:
                        nc.scalar.copy(dst, t2[:, hh, :])

        # ---- Phase K: project+norm+rope+transpose K for all 16 chunks ----
        with ExitStack() as ph:
            wpool = ph.enter_context(tc.tile_pool(name="wpk", bufs=1))
            xpool = ph.enter_context(tc.tile_pool(name="xk", bufs=3))
            stg = ph.enter_context(tc.tile_pool(name="stgk", bufs=1))
            pspool = ph.enter_context(tc.tile_pool(name="psk", bufs=2, space="PSUM"))
            tppool = ph.enter_context(tc.tile_pool(name="tpk", bufs=1, space="PSUM"))
            tp2pool = ph.enter_context(tc.tile_pool(name="tp2k", bufs=1, space="PSUM"))
            w_sb = load_w(wpool, wk, "wk_sb")
            b_sb = load_bias(wpool, bk, "bk_sb") if use_bk else None
            for ci in range(NCK):
                ps = proj_chunk(xb, ci, w_sb, b_sb, xpool, tppool, pspool)
                rot = norm_rope(ps, cosk_sb[:, ci, :], sink_sb[:, ci, :], knw_b, stg)
                transpose_heads(rot, kT, ci, tp2pool)

        # ---- Phase Q: same for the core's own 8 chunks ----
        with ExitStack() as ph:
            wpool = ph.enter_context(tc.tile_pool(name="wpq", bufs=1))
            xpool = ph.enter_context(tc.tile_pool(name="xq", bufs=3))
            stg = ph.enter_context(tc.tile_pool(name="stgq", bufs=1))
            pspool = ph.enter_context(tc.tile_pool(name="psq", bufs=2, space="PSUM"))
            tppool = ph.enter_context(tc.tile_pool(name="tpq", bufs=1, space="PSUM"))
            tp2pool = ph.enter_context(tc.tile_pool(name="tp2q", bufs=1, space="PSUM"))
            w_sb = load_w(wpool, wq, "wq_sb")
            b_sb = load_bias(wpool, bq, "bq_sb") if use_bq else None
            for ci in range(NCQ):
                ps = proj_chunk(xq, ci, w_sb, b_sb, xpool, tppool, pspool)
                rot = norm_rope(ps, cosq_sb[:, ci, :], sinq_sb[:, ci, :], qnw_b, stg)
                transpose_heads(rot, qT, ci, tp2pool)

        # ---- Phase V: project V for all 16 chunks, stage to DRAM ----
        with ExitStack() as ph:
            wpool = ph.enter_context(tc.tile_pool(name="wpv", bufs=1))
            xpool = ph.enter_context(tc.tile_pool(name="xv", bufs=3))
            stg = ph.enter_context(tc.tile_pool(name="stgv", bufs=2))
            pspool = ph.enter_context(tc.tile_pool(name="psv", bufs=2, space="PSUM"))
            tppool = ph.enter_context(tc.tile_pool(name="tpv", bufs=2, space="PSUM"))
            w_sb = load_w(wpool, wv, "wv_sb")
            b_sb = load_bias(wpool, bv, "bv_sb") if use_bv else None
            for ci in range(NCK):
                ps = proj_chunk(xb, ci, w_sb, b_sb, xpool, tppool, pspool)
                vsb = stg.tile([P, H, HD], F32, tag="vsb")
                nc.scalar.copy(vsb, ps.rearrange("p (h d) -> p h d", h=H))
                for h in range(H):
                    nc.sync.dma_start(
                        out=vstage[h, ci * P : (ci + 1) * P, 0:HD],
                        in_=vsb[:, h, :],
                    )

        # ---- Phase attention + out projection ----
        with ExitStack() as ph:
            wpool = ph.enter_context(tc.tile_pool(name="wpo", bufs=1))
            wout_sb = load_w(wpool, wout, "wout_sb")
            attnT = wpool.tile([P, H // 2, LQ], FR, tag="attnT")
            with ExitStack() as ph2:
                ppool = ph2.enter_context(tc.tile_pool(name="pt", bufs=2))
                vcpool = ph2.enter_context(tc.tile_pool(name="vc", bufs=2))
                invpool = ph2.enter_context(tc.tile_pool(name="invp", bufs=2))
                upool = ph2.enter_context(tc.tile_pool(name="ups", bufs=2, space="PSUM"))
                spool = ph2.enter_context(tc.tile_pool(name="sps", bufs=2, space="PSUM"))
                for h in range(H):
                    pi, po = h // 2, (h % 2) * HD
                    U = upool.tile([HD + 1, LQ], F32, tag="U")
                    for ci in range(NCK):
                        sT = spool.tile([P, LQ], F32, tag="sT")
                        kslice = _fr(kT[po : po + HD, pi, ci * P : (ci + 1) * P])
                        for n0 in range(0, LQ, 512):
                            nc.tensor.matmul(
                                sT[:, n0 : n0 + 512],
                                kslice,
                                _fr(qT[po : po + HD, pi, n0 : n0 + 512]),
                                start=True,
                                stop=True,
                            )
                        Pt = ppool.tile([P, LQ], FR, tag="Pt")
                        nc.scalar.activation(Pt, sT, AF.Exp, scale=HD ** -0.5)
                        vc = vcpool.tile([P, HD + 1], FR, tag="vc")
                        nc.sync.dma_start(
                            out=vc,
                            in_=vstage[h, ci * P : (ci + 1) * P, :].bitcast(FR),
                        )
                        for n0 in range(0, LQ, 512):
                            nc.tensor.matmul(
                                U[:, n0 : n0 + 512],
                                _fr(vc),
                                _fr(Pt[:, n0 : n0 + 512]),
                                start=(ci == 0),
                                stop=(ci == NCK - 1),
                            )
                    nc.vector.tensor_copy(attnT[po : po + HD, pi, :], U[0:HD, :])
                    inv = invpool.tile([1, LQ], F32, tag="inv")
                    nc.vector.reciprocal(inv, U[HD : HD + 1, :])
                    nc.sync.dma_start(out=invstage[h, :], in_=inv)

            with ExitStack() as ph2:
                bcpool = ph2.enter_context(tc.tile_pool(name="bcp", bufs=2))
                opool = ph2.enter_context(tc.tile_pool(name="ops", bufs=2, space="PSUM"))
                obpool = ph2.enter_context(tc.tile_pool(name="obp", bufs=2))
                for pi in range(H // 2):
                    bc = bcpool.tile([P, LQ], F32, tag="bc")
                    for hh in range(2):
                        src = invstage[2 * pi + hh, :]
                        nc.sync.dma_start(
                            out=bc[hh * HD : (hh + 1) * HD, :],
                            in_=bass.AP(tensor=src.tensor, offset=src.offset,
                                        ap=[[0, HD], [1, LQ]]),
                        )
                    nc.vector.tensor_mul(attnT[:, pi, :], attnT[:, pi, :], bc)
                for cj in range(NCQ):
                    pso = opool.tile([P, D], F32, tag="pso")
                    for n0 in range(0, D, 512):
                        for j in range(NDC):
                            nc.tensor.matmul(
                                pso[:, n0 : n0 + 512],
                                _fr(attnT[:, j, cj * P : (cj + 1) * P]),
                                _fr(wout_sb[:, j, n0 : n0 + 512]),
                                start=(j == 0),
                                stop=(j == NDC - 1),
                            )
                    ob = obpool.tile([P, D], F32, tag="ob")
                    if use_bout:
                        nc.vector.tensor_add(ob, pso, bout_b)
                    else:
                        nc.vector.tensor_copy(ob, pso)
                    nc.sync.dma_start(out=out[cj * P : (cj + 1) * P, :], in_=ob)

    return nc


_PROGRAM_CACHE = {}


def _get_program(flags):
    if flags not in _PROGRAM_CACHE:
        _patch_tile_drain()
        _PROGRAM_CACHE[flags] = _build_program(*flags)
    return _PROGRAM_CACHE[flags]


def _rope_tables():
    pos = np.arange(L, dtype=np.float32)
    inv_freq = (1.0 / (THETA ** (np.arange(0, HD, 2, dtype=np.float32) / HD))).astype(
        np.float32
    )
    ang = pos[:, None] * inv_freq[None, :]
    return np.cos(ang).astype(np.float32), np.sin(ang).astype(np.float32)


def _make_in_maps(x, Wqkv, bqkv, qn_w, kn_w, Wout, bout, flags):
    use_bq, use_bk, use_bv, use_bout, use_qnw, use_knw = flags
    cos, sin = _rope_tables()
    wq = np.ascontiguousarray(Wqkv[:, 0:D])
    wk = np.ascontiguousarray(Wqkv[:, D : 2 * D])
    wv = np.ascontiguousarray(Wqkv[:, 2 * D : 3 * D])
    in_maps = []
    for c in range(N_CORES):
        b, half = c // 2, c % 2
        m = {
            "xb": np.ascontiguousarray(x[b]),
            "xq": np.ascontiguousarray(x[b, half * LQ : (half + 1) * LQ, :]),
            "wq": wq,
            "wk": wk,
            "wv": wv,
            "wout": np.ascontiguousarray(Wout),
            "cosk": cos,
            "sink": sin,
            "cosq": np.ascontiguousarray(cos[half * LQ : (half + 1) * LQ]),
            "sinq": np.ascontiguousarray(sin[half * LQ : (half + 1) * LQ]),
        }
        if use_bq:
            m["bq"] = np.ascontiguousarray(bqkv[0:D]).reshape(1, D)
        if use_bk:
            m["bk"] = np.ascontiguousarray(bqkv[D : 2 * D]).reshape(1, D)
        if use_bv:
            m["bv"] = np.ascontiguousarray(bqkv[2 * D : 3 * D]).reshape(1, D)
        if use_bout:
            m["bout"] = np.ascontiguousarray(bout).reshape(1, D)
        if use_qnw:
            m["qnw"] = np.ascontiguousarray(qn_w).reshape(1, HD)
        if use_knw:
            m["knw"] = np.ascontiguousarray(kn_w).reshape(1, HD)
        in_maps.append(m)
    return in_maps


def _flags_for(bqkv, qn_w, kn_w, bout):
    return (
        bool(np.any(bqkv[0:D])),
        bool(np.any(bqkv[D : 2 * D])),
        bool(np.any(bqkv[2 * D : 3 * D])),
        bool(np.any(bout)),
        bool(np.any(qn_w != 1.0)),
        bool(np.any(kn_w != 1.0)),
    )


def _assemble(results):
    out = np.empty((B, L, D), dtype=np.float32)
    for c in range(N_CORES):
        b, half = c // 2, c % 2
        out[b, half * LQ : (half + 1) * LQ, :] = results[c]["out"]
    return out


def kernel(x, Wqkv, bqkv, qn_w, kn_w, Wout, bout, _trace=False):
    x = np.asarray(x, dtype=np.float32)
    Wqkv = np.asarray(Wqkv, dtype=np.float32)
    bqkv = np.asarray(bqkv, dtype=np.float32)
    qn_w = np.asarray(qn_w, dtype=np.float32)
    kn_w = np.asarray(kn_w, dtype=np.float32)
    Wout = np.asarray(Wout, dtype=np.float32)
    bout = np.asarray(bout, dtype=np.float32)

    flags = _flags_for(bqkv, qn_w, kn_w, bout)
    nc = _get_program(flags)
    if not getattr(nc, "_waits_split", False):
        _split_waits(nc)
        nc._waits_split = True
    in_maps = _make_in_maps(x, Wqkv, bqkv, qn_w, kn_w, Wout, bout, flags)
    res = bass_utils.run_bass_kernel_spmd(
        nc, in_maps, core_ids=list(range(N_CORES))
    )
    out = _assemble(res.results)
    if _trace:
        return out, res
    return out


# revision 12
# speedup vs baseline: 6.2325x; 6.2325x over previous
"""DiT attention block (QKV proj + QK-RMSNorm + RoPE + softmax attention + out proj)
as a Bass/Tile kernel for 8 Trainium2 NeuronCores.

Sharding (zero cross-core communication):
  core c -> batch b = c//2, sequence half = c%2.
  Each core computes output rows [half*1024, half*1024+1024) of batch b:
    - K, V are computed for the full 2048-row sequence of batch b (duplicated
      within a core pair); Q only for the core's own 1024 rows.
    - attention + out-proj for the core's 1024 query rows.
  Host concatenates the 8 disjoint row blocks into the full [4, 2048, 1024].

Layout strategy per core:
  - x chunks are transposed on the tensor engine (via identity matmul) to get
    the contraction dim (D) on partitions for the QKV projections.
  - Q/K are projected in "natural" [l, d] layout, RMS-normed + roped there
    (free-dim reduces), then transposed per head to q^T/k^T [hd, l], packed
    two heads per 128 partitions.
  - S^T = k^T.T @ q^T is computed per (head, 128-row lk chunk) into PSUM
    [128 lk, 1024 lq]; ScalarE applies exp(0.125*S) into SBUF; the AV matmul
    uses [v_h | ones] as the stationary operand so PSUM row 64 accumulates the
    softmax denominator for free.  attn^T is normalized afterwards and used
    directly as the stationary operand of the out projection.
  - Matmuls run in float32r (fp32 bits, FP22 multiplies) which streams at
    bf16 rate for moving free dims >= 256.
"""

import sys

if "/opt/trn_rl_repo" not in sys.path:
    sys.path.insert(0, "/opt/trn_rl_repo")

from contextlib import ExitStack

import numpy as np

import concourse.bass as bass
import concourse.tile as tile
from concourse import mybir, bass_utils
from concourse.masks import make_identity
from concourse.vector_clock import ScopedClock, VectorClock

B, L, D, H = 4, 2048, 1024, 16
HD = D // H          # 64
HHD = HD // 2        # 32
EPS = 1e-6
THETA = 10000.0
N_CORES = 8
LQ = L // 2          # query rows per core
P = 128
NCK = L // P         # 16 l-chunks for K/V
NCQ = LQ // P        # 8 l-chunks for Q
NDC = D // P         # 8 contraction chunks
F32 = mybir.dt.float32
FR = mybir.dt.float32r
AF = mybir.ActivationFunctionType


def _patch_tile_drain():
    """This container's walrus rejects >1 sem wait per instruction.
    Tile's kernel-tail drain waits on every active proc at once; split those
    waits across single-wait NOPs on SP so the drain itself needs none."""
    if getattr(tile.TileContext, "_drain_split_patched", False):
        return

    def _patched(self, tick_clock, wait_clock):
        vc = tick_clock.global_clock
        n = len(vc)
        cur = VectorClock([0] * n)
        for proc in range(n):
            t = vc[proc]
            if t > 0:
                nop = self.nc.sync.nop(hint=f"drainsplit_{proc}", nofuse=True)
                req = VectorClock([0] * n)
                req.require_at_least(proc, t)
                wait_clock.add_sem_waits(
                    nop.ins, ScopedClock({None: req}), ScopedClock({None: cur.copy()})
                )
                cur.require_at_least(proc, t)
        drain_inst = self.nc.sync.drain()
        wait_clock.add_sem_waits(
            drain_inst.ins, ScopedClock({None: vc}), ScopedClock({None: cur})
        )
        self.nc.all_engine_barrier()
        popped = self.nc._tile_sem_poison_stack.pop()
        assert popped is self._sem_poison
        self.nc.clear_and_free_semaphores(list(self.sems.allocated().values()))
        self.nc.all_engine_barrier()

    tile.TileContext._drain_and_barrier = _patched
    tile.TileContext._drain_split_patched = True


def _split_waits(nc, maxw=1):
    """This walrus build allows at most one sync wait per instruction.
    After Tile has assigned semaphores, hoist excess waits onto NOPs
    inserted just before the over-subscribed instruction (same engine,
    same block) — semantically identical, since all waits must clear
    before the instruction executes either way."""
    nid = 0
    for fn in nc.m.functions:
        for bb in fn.blocks:
            insts = list(bb.instructions)
            new = []
            changed = False
            for inst in insts:
                si = inst.sync_info
                if si is not None and si.on_wait is not None and len(si.on_wait) > maxw:
                    waits = list(si.on_wait)
                    extra, keep = waits[:-maxw], waits[-maxw:]
                    for i in range(0, len(extra), maxw):
                        nid += 1
                        new.append(mybir.InstNoOp(
                            name=f"I-wsplit-{nid}", engine=inst.engine,
                            sync_info=mybir.SyncInfo(
                                on_wait=extra[i : i + maxw], on_update=[]),
                        ))
                    inst.sync_info = mybir.SyncInfo(
                        on_wait=keep, on_update=list(si.on_update))
                    changed = True
                new.append(inst)
            if changed:
                bb.instructions = new


def _bcast_free(ap, repeat, at):
    """Insert a step-0 free dim of size `repeat` at free-dim position `at`
    (0 = right after the partition dim)."""
    new = ap.copy()
    new.ap = new.ap[: 1 + at] + [[0, repeat]] + new.ap[1 + at :]
    return new


def _fr(ap):
    if ap.dtype == FR:
        return ap
    return ap.bitcast(FR)


def _build_program(use_bq, use_bk, use_bv, use_bout, use_qnw, use_knw):
    nc = bass.Bass("TRN2", target_bir_lowering=False, debug=False,
                   num_devices=N_CORES)

    xb = nc.dram_tensor("xb", [L, D], F32, kind="ExternalInput").ap()
    xq = nc.dram_tensor("xq", [LQ, D], F32, kind="ExternalInput").ap()
    wq = nc.dram_tensor("wq", [D, D], F32, kind="ExternalInput").ap()
    wk = nc.dram_tensor("wk", [D, D], F32, kind="ExternalInput").ap()
    wv = nc.dram_tensor("wv", [D, D], F32, kind="ExternalInput").ap()
    wout = nc.dram_tensor("wout", [D, D], F32, kind="ExternalInput").ap()
    cosk = nc.dram_tensor("cosk", [L, HHD], F32, kind="ExternalInput").ap()
    sink = nc.dram_tensor("sink", [L, HHD], F32, kind="ExternalInput").ap()
    cosq = nc.dram_tensor("cosq", [LQ, HHD], F32, kind="ExternalInput").ap()
    sinq = nc.dram_tensor("sinq", [LQ, HHD], F32, kind="ExternalInput").ap()
    bq = bk = bv = bo = qnw = knw = None
    if use_bq:
        bq = nc.dram_tensor("bq", [1, D], F32, kind="ExternalInput").ap()
    if use_bk:
        bk = nc.dram_tensor("bk", [1, D], F32, kind="ExternalInput").ap()
    if use_bv:
        bv = nc.dram_tensor("bv", [1, D], F32, kind="ExternalInput").ap()
    if use_bout:
        bo = nc.dram_tensor("bout", [1, D], F32, kind="ExternalInput").ap()
    if use_qnw:
        qnw = nc.dram_tensor("qnw", [1, HD], F32, kind="ExternalInput").ap()
    if use_knw:
        knw = nc.dram_tensor("knw", [1, HD], F32, kind="ExternalInput").ap()
    out = nc.dram_tensor("out", [LQ, D], F32, kind="ExternalOutput").ap()

    with tile.TileContext(nc) as tc, ExitStack() as ctx:
        pers = ctx.enter_context(tc.tile_pool(name="pers", bufs=1))
        dpool = ctx.enter_context(tc.tile_pool(name="dram", bufs=1, space="DRAM"))

        ident = pers.tile([P, P], F32, tag="ident")
        make_identity(nc, ident)

        cosk_sb = pers.tile([P, NCK, HHD], F32, tag="cosk")
        sink_sb = pers.tile([P, NCK, HHD], F32, tag="sink")
        cosq_sb = pers.tile([P, NCQ, HHD], F32, tag="cosq")
        sinq_sb = pers.tile([P, NCQ, HHD], F32, tag="sinq")
        nc.sync.dma_start(out=cosk_sb, in_=cosk.rearrange("(c p) f -> p c f", p=P))
        nc.sync.dma_start(out=sink_sb, in_=sink.rearrange("(c p) f -> p c f", p=P))
        nc.sync.dma_start(out=cosq_sb, in_=cosq.rearrange("(c p) f -> p c f", p=P))
        nc.sync.dma_start(out=sinq_sb, in_=sinq.rearrange("(c p) f -> p c f", p=P))

        # packed transposed activations: pair tile p holds head 2p in
        # partitions 0:64 and head 2p+1 in partitions 64:128.
        kT = pers.tile([P, H // 2, L], FR, tag="kT")
        qT = pers.tile([P, H // 2, LQ], FR, tag="qT")

        ones16 = pers.tile([P, H], F32, tag="ones16")
        nc.vector.memset(ones16, 1.0)
        eps_sb = pers.tile([P, 1], F32, tag="eps")
        nc.vector.memset(eps_sb, EPS)
        ones1 = None
        if use_bq or use_bk or use_bv:
            ones1f = pers.tile([1, P], F32, tag="ones1f")
            nc.vector.memset(ones1f, 1.0)
            ones1 = pers.tile([1, P], FR, tag="ones1")
            nc.vector.tensor_copy(ones1, ones1f)
        qnw_b = knw_b = bout_b = None
        if use_qnw:
            qnw_b = pers.tile([P, HD], F32, tag="qnw_b")
            nc.sync.dma_start(
                out=qnw_b,
                in_=bass.AP(tensor=qnw.tensor, offset=qnw.offset,
                            ap=[[0, P], [1, HD]]),
            )
        if use_knw:
            knw_b = pers.tile([P, HD], F32, tag="knw_b")
            nc.sync.dma_start(
                out=knw_b,
                in_=bass.AP(tensor=knw.tensor, offset=knw.offset,
                            ap=[[0, P], [1, HD]]),
            )
        if use_bout:
            bout_b = pers.tile([P, D], F32, tag="bout_b")
            nc.sync.dma_start(
                out=bout_b,
                in_=bass.AP(tensor=bo.tensor, offset=bo.offset,
                            ap=[[0, P], [1, D]]),
            )

        # DRAM staging: v with a ones column appended (AV stationary operand),
        # and per-head softmax denominators for the broadcast bounce.
        vstage = dpool.tile([L, D], F32, tag="vstage")
        invstage = dpool.tile([H, LQ], F32, tag="invstage")

        def load_w(pool, w_dram, tag):
            w_sb = pool.tile([P, NDC, D], FR, tag=tag)
            nc.sync.dma_start(
                out=w_sb,
                in_=w_dram.rearrange("(j p) n -> p j n", p=P).bitcast(FR),
            )
            return w_sb

        def load_bias(pool, b_dram, tag):
            b_sb = pool.tile([1, D], FR, tag=tag)
            nc.sync.dma_start(out=b_sb, in_=b_dram.bitcast(FR))
            return b_sb

        def proj_chunk(x_dram, ci, w_sb, b_sb, xpool, tppool, pspool):
            """Project one 128-row chunk: returns PSUM [128, D] = x_chunk @ W (+b)."""
            xc = xpool.tile([P, D], F32, tag="xc")
            nc.sync.dma_start(out=xc, in_=x_dram[ci * P : (ci + 1) * P, :])
            xt = xpool.tile([P, NDC, P], FR, tag="xt")
            for half in range(2):
                tp = tppool.tile([P, NDC * P // 2], F32, tag="tp")
                for jj in range(NDC // 2):
                    j = half * (NDC // 2) + jj
                    nc.tensor.transpose(
                        tp[:, jj * P : (jj + 1) * P], xc[:, j * P : (j + 1) * P],
                        ident,
                    )
                dst = xt[:, half * (NDC // 2) : (half + 1) * (NDC // 2), :]
                tps = tp.rearrange("p (j q) -> p j q", j=NDC // 2)
                if half == 0:
                    nc.scalar.copy(dst, tps)
                else:
                    nc.vector.tensor_copy(dst, tps)
            ps = pspool.tile([P, D], F32, tag="ps")
            for n0 in range(0, D, 512):
                for j in range(NDC):
                    nc.tensor.matmul(
                        ps[:, n0 : n0 + 512],
                        _fr(xt[:, j, :]),
                        _fr(w_sb[:, j, n0 : n0 + 512]),
                        start=(j == 0),
                        stop=(j == NDC - 1 and b_sb is None),
                    )
                if b_sb is not None:
                    nc.tensor.matmul(
                        ps[:, n0 : n0 + 512],
                        _fr(ones1),
                        _fr(b_sb[:, n0 : n0 + 512]),
                        start=False,
                        stop=True,
                    )
            return ps

        def norm_rope(ps, cos_ap, sin_ap, nw_b, stg):
            """RMSNorm over head_dim + rotary embed, from PSUM [128, D] in
            natural layout; returns SBUF tile [128, H, HD]."""
            sq = stg.tile([P, D], F32, tag="big")
            nc.scalar.activation(sq, ps, AF.Square)
            ss = stg.tile([P, H], F32, tag="ss")
            nc.vector.tensor_reduce(
                ss, sq.rearrange("p (h d) -> p h d", h=H),
                axis=mybir.AxisListType.X, op=mybir.AluOpType.add,
            )
            inv = stg.tile([P, H], F32, tag="inv")
            nc.scalar.activation(inv, ss, AF.Sqrt, scale=1.0 / HD, bias=eps_sb)
            nc.vector.reciprocal(inv, inv)
            ps3 = ps.rearrange("p (h d) -> p h d", h=H)
            kn = stg.tile([P, H, HD], F32, tag="kn")
            nc.vector.tensor_mul(kn, ps3, _bcast_free(inv, HD, 1))
            if nw_b is not None:
                nc.vector.tensor_mul(kn, kn, _bcast_free(nw_b, H, 0))
            t1 = kn[:, :, 0:HHD]
            t2 = kn[:, :, HHD:HD]
            cosc = _bcast_free(cos_ap, H, 0)
            sinc = _bcast_free(sin_ap, H, 0)
            ra = stg.tile([P, H, HHD], F32, tag="ra")
            rb = stg.tile([P, H, HHD], F32, tag="rb")
            rot = stg.tile([P, H, HD], F32, tag="big")
            nc.vector.tensor_mul(ra, t1, cosc)
            nc.vector.tensor_mul(rb, t2, sinc)
            nc.vector.tensor_sub(rot[:, :, 0:HHD], ra, rb)
            nc.vector.tensor_mul(ra, t1, sinc)
            nc.vector.tensor_mul(rb, t2, cosc)
            nc.vector.tensor_add(rot[:, :, HHD:HD], ra, rb)
            return rot

        def transpose_heads(rot, dstT, ci, tp2pool):
            """Per-head PE transpose of [128, HD] -> [HD, 128], packed into
            dstT[(h%2)*64:(h%2)*64+64, h//2, ci*128:ci*128+128]."""
            # group = (parity e, pair-half g): 4 heads h = 2*(4g+pi)+e; one
            # batched copy per group into dstT[e*64:(e+1)*64, 4g:4g+4, ci].
            for e in range(2):
                for g in range(2):
                    t2 = tp2pool.tile([HD, 4, P], F32, tag="t2")
                    for pp in range(4):
                        h = 2 * (4 * g + pp) + e
                        nc.tensor.transpose(t2[:, pp, :], rot[:, h, :], ident)
                    dst = dstT[e * HD : (e + 1) * HD, 4 * g : 4 * g + 4,
                               ci * P : (ci + 1) * P]
                    if e == 0:
                        nc.vector.tensor_copy(dst, t2)
                    else:
                        nc.scalar.copy(dst, t2)

        # ---- Phase K: project+norm+rope+transpose K for all 16 chunks ----
        with ExitStack() as ph:
            wpool = ph.enter_context(tc.tile_pool(name="wpk", bufs=1))
            xpool = ph.enter_context(tc.tile_pool(name="xk", bufs=3))
            stg = ph.enter_context(tc.tile_pool(name="stgk", bufs=2))
            pspool = ph.enter_context(tc.tile_pool(name="psk", bufs=2, space="PSUM"))
            tppool = ph.enter_context(tc.tile_pool(name="tpk", bufs=2, space="PSUM"))
            tp2pool = ph.enter_context(tc.tile_pool(name="tp2k", bufs=2, space="PSUM"))
            w_sb = load_w(wpool, wk, "wk_sb")
            b_sb = load_bias(wpool, bk, "bk_sb") if use_bk else None
            for ci in range(NCK):
                ps = proj_chunk(xb, ci, w_sb, b_sb, xpool, tppool, pspool)
                rot = norm_rope(ps, cosk_sb[:, ci, :], sink_sb[:, ci, :], knw_b, stg)
                transpose_heads(rot, kT, ci, tp2pool)

        # ---- Phase Q: same for the core's own 8 chunks ----
        with ExitStack() as ph:
            wpool = ph.enter_context(tc.tile_pool(name="wpq", bufs=1))
            xpool = ph.enter_context(tc.tile_pool(name="xq", bufs=3))
            stg = ph.enter_context(tc.tile_pool(name="stgq", bufs=2))
            pspool = ph.enter_context(tc.tile_pool(name="psq", bufs=2, space="PSUM"))
            tppool = ph.enter_context(tc.tile_pool(name="tpq", bufs=2, space="PSUM"))
            tp2pool = ph.enter_context(tc.tile_pool(name="tp2q", bufs=2, space="PSUM"))
            w_sb = load_w(wpool, wq, "wq_sb")
            b_sb = load_bias(wpool, bq, "bq_sb") if use_bq else None
            for ci in range(NCQ):
                ps = proj_chunk(xq, ci, w_sb, b_sb, xpool, tppool, pspool)
                rot = norm_rope(ps, cosq_sb[:, ci, :], sinq_sb[:, ci, :], qnw_b, stg)
                transpose_heads(rot, qT, ci, tp2pool)

        # ---- Phase V: project V for all 16 chunks, stage to DRAM ----
        with ExitStack() as ph:
            wpool = ph.enter_context(tc.tile_pool(name="wpv", bufs=1))
            xpool = ph.enter_context(tc.tile_pool(name="xv", bufs=3))
            stg = ph.enter_context(tc.tile_pool(name="stgv", bufs=2))
            pspool = ph.enter_context(tc.tile_pool(name="psv", bufs=2, space="PSUM"))
            tppool = ph.enter_context(tc.tile_pool(name="tpv", bufs=2, space="PSUM"))
            w_sb = load_w(wpool, wv, "wv_sb")
            b_sb = load_bias(wpool, bv, "bv_sb") if use_bv else None
            for ci in range(NCK):
                ps = proj_chunk(xb, ci, w_sb, b_sb, xpool, tppool, pspool)
                vsb = stg.tile([P, D], F32, tag="vsb")
                nc.scalar.copy(vsb[:, 0 : D // 2], ps[:, 0 : D // 2])
                nc.vector.tensor_copy(vsb[:, D // 2 : D], ps[:, D // 2 : D])
                nc.sync.dma_start(
                    out=vstage[ci * P : (ci + 1) * P, :], in_=vsb
                )

        # ---- Phase attention + out projection ----
        with ExitStack() as ph:
            wpool = ph.enter_context(tc.tile_pool(name="wpo", bufs=1))
            wout_sb = load_w(wpool, wout, "wout_sb")
            attnT = wpool.tile([P, H // 2, LQ], FR, tag="attnT")
            with ExitStack() as ph2:
                ppool = ph2.enter_context(tc.tile_pool(name="pt", bufs=2))
                vcpool = ph2.enter_context(tc.tile_pool(name="vc", bufs=3))
                invpool = ph2.enter_context(tc.tile_pool(name="invp", bufs=2))
                upool = ph2.enter_context(tc.tile_pool(name="ups", bufs=2, space="PSUM"))
                spool = ph2.enter_context(tc.tile_pool(name="sps", bufs=2, space="PSUM"))
                def s_chunk(h, ci):
                    pi, po = h // 2, (h % 2) * HD
                    sT = spool.tile([P, LQ], F32, tag="sT")
                    kslice = kT[po : po + HD, pi, ci * P : (ci + 1) * P]
                    for n0 in range(0, LQ, 512):
                        nc.tensor.matmul(
                            sT[:, n0 : n0 + 512],
                            kslice,
                            qT[po : po + HD, pi, n0 : n0 + 512],
                            start=True,
                            stop=True,
                        )
                    return sT

                def av_chunk(h, ci, sT, U):
                    Pt = ppool.tile([P, LQ], FR, tag="Pt")
                    nc.scalar.activation(Pt, sT, AF.Exp, scale=HD ** -0.5)
                    vc = vcpool.tile([P, HD + 1], FR, tag="vc")
                    nc.gpsimd.dma_start(
                        out=vc[:, 0:HD],
                        in_=vstage[ci * P : (ci + 1) * P,
                                   (h * HD) : (h + 1) * HD].bitcast(FR),
                    )
                    nc.gpsimd.memset(vc[:, HD : HD + 1], 1.0)
                    for n0 in range(0, LQ, 512):
                        nc.tensor.matmul(
                            U[:, n0 : n0 + 512],
                            vc,
                            Pt[:, n0 : n0 + 512],
                            start=(ci == 0),
                            stop=(ci == NCK - 1),
                        )

                for h in range(H):
                    pi, po = h // 2, (h % 2) * HD
                    U = upool.tile([HD + 1, LQ], F32, tag="U")
                    sT_prev = s_chunk(h, 0)
                    for ci in range(1, NCK):
                        sT_cur = s_chunk(h, ci)
                        av_chunk(h, ci - 1, sT_prev, U)
                        sT_prev = sT_cur
                    av_chunk(h, NCK - 1, sT_prev, U)
                    nc.vector.tensor_copy(attnT[po : po + HD, pi, :], U[0:HD, :])
                    inv = invpool.tile([1, LQ], F32, tag="inv")
                    nc.vector.reciprocal(inv, U[HD : HD + 1, :])
                    nc.sync.dma_start(out=invstage[h, :], in_=inv)

            with ExitStack() as ph2:
                bcpool = ph2.enter_context(tc.tile_pool(name="bcp", bufs=2))
                opool = ph2.enter_context(tc.tile_pool(name="ops", bufs=2, space="PSUM"))
                obpool = ph2.enter_context(tc.tile_pool(name="obp", bufs=2))
                for pi in range(H // 2):
                    bc = bcpool.tile([P, LQ], F32, tag="bc")
                    for hh in range(2):
                        src = invstage[2 * pi + hh, :]
                        nc.sync.dma_start(
                            out=bc[hh * HD : (hh + 1) * HD, :],
                            in_=bass.AP(tensor=src.tensor, offset=src.offset,
                                        ap=[[0, HD], [1, LQ]]),
                        )
                    nc.vector.tensor_mul(attnT[:, pi, :], attnT[:, pi, :], bc)
                for cj in range(NCQ):
                    pso = opool.tile([P, D], F32, tag="pso")
                    for n0 in range(0, D, 512):
                        for j in range(NDC):
                            nc.tensor.matmul(
                                pso[:, n0 : n0 + 512],
                                _fr(attnT[:, j, cj * P : (cj + 1) * P]),
                                _fr(wout_sb[:, j, n0 : n0 + 512]),
                                start=(j == 0),
                                stop=(j == NDC - 1),
                            )
                    ob = obpool.tile([P, D], F32, tag="ob")
                    if use_bout:
                        nc.vector.tensor_add(ob, pso, bout_b)
                    else:
                        nc.vector.tensor_copy(ob, pso)
                    nc.sync.dma_start(out=out[cj * P : (cj + 1) * P, :], in_=ob)

    return nc


_PROGRAM_CACHE = {}


def _get_program(flags):
    if flags not in _PROGRAM_CACHE:
        _patch_tile_drain()
        _PROGRAM_CACHE[flags] = _build_program(*flags)
    return _PROGRAM_CACHE[flags]


def _rope_tables():
    pos = np.arange(L, dtype=np.float32)
    inv_freq = (1.0 / (THETA ** (np.arange(0, HD, 2, dtype=np.float32) / HD))).astype(
        np.float32
    )
    ang = pos[:, None] * inv_freq[None, :]
    return np.cos(ang).astype(np.float32), np.sin(ang).astype(np.float32)


def _make_in_maps(x, Wqkv, bqkv, qn_w, kn_w, Wout, bout, flags):
    use_bq, use_bk, use_bv, use_bout, use_qnw, use_knw = flags
    cos, sin = _rope_tables()
    wq = np.ascontiguousarray(Wqkv[:, 0:D])
    wk = np.ascontiguousarray(Wqkv[:, D : 2 * D])
    wv = np.ascontiguousarray(Wqkv[:, 2 * D : 3 * D])
    in_maps = []
    for c in range(N_CORES):
        b, half = c // 2, c % 2
        m = {
            "xb": np.ascontiguousarray(x[b]),
            "xq": np.ascontiguousarray(x[b, half * LQ : (half + 1) * LQ, :]),
            "wq": wq,
            "wk": wk,
            "wv": wv,
            "wout": np.ascontiguousarray(Wout),
            "cosk": cos,
            "sink": sin,
            "cosq": np.ascontiguousarray(cos[half * LQ : (half + 1) * LQ]),
            "sinq": np.ascontiguousarray(sin[half * LQ : (half + 1) * LQ]),
        }
        if use_bq:
            m["bq"] = np.ascontiguousarray(bqkv[0:D]).reshape(1, D)
        if use_bk:
            m["bk"] = np.ascontiguousarray(bqkv[D : 2 * D]).reshape(1, D)
        if use_bv:
            m["bv"] = np.ascontiguousarray(bqkv[2 * D : 3 * D]).reshape(1, D)
        if use_bout:
            m["bout"] = np.ascontiguousarray(bout).reshape(1, D)
        if use_qnw:
            m["qnw"] = np.ascontiguousarray(qn_w).reshape(1, HD)
        if use_knw:
            m["knw"] = np.ascontiguousarray(kn_w).reshape(1, HD)
        in_maps.append(m)
    return in_maps


def _flags_for(bqkv, qn_w, kn_w, bout):
    return (
        bool(np.any(bqkv[0:D])),
        bool(np.any(bqkv[D : 2 * D])),
        bool(np.any(bqkv[2 * D : 3 * D])),
        bool(np.any(bout)),
        bool(np.any(qn_w != 1.0)),
        bool(np.any(kn_w != 1.0)),
    )


def _assemble(results):
    out = np.empty((B, L, D), dtype=np.float32)
    for c in range(N_CORES):
        b, half = c // 2, c % 2
        out[b, half * LQ : (half + 1) * LQ, :] = results[c]["out"]
    return out


def kernel(x, Wqkv, bqkv, qn_w, kn_w, Wout, bout, _trace=False):
    x = np.asarray(x, dtype=np.float32)
    Wqkv = np.asarray(Wqkv, dtype=np.float32)
    bqkv = np.asarray(bqkv, dtype=np.float32)
    qn_w = np.asarray(qn_w, dtype=np.float32)
    kn_w = np.asarray(kn_w, dtype=np.float32)
    Wout = np.asarray(Wout, dtype=np.float32)
    bout = np.asarray(bout, dtype=np.float32)

    flags = _flags_for(bqkv, qn_w, kn_w, bout)
    nc = _get_program(flags)
    if not getattr(nc, "_waits_split", False):
        _split_waits(nc)
        nc._waits_split = True
    in_maps = _make_in_maps(x, Wqkv, bqkv, qn_w, kn_w, Wout, bout, flags)
    res = bass_utils.run_bass_kernel_spmd(
        nc, in_maps, core_ids=list(range(N_CORES))
    )
    out = _assemble(res.results)
    if _trace:
        return out, res
    return out


# revision 14
# speedup vs baseline: 8.5282x; 1.3684x over previous
"""DiT attention block (QKV proj + QK-RMSNorm + RoPE + softmax attention + out proj)
as a Bass/Tile kernel for 8 Trainium2 NeuronCores.

Sharding (zero cross-core communication):
  core c -> batch b = c//2, sequence half = c%2.
  Each core computes output rows [half*1024, half*1024+1024) of batch b:
    - K, V are computed for the full 2048-row sequence of batch b (duplicated
      within a core pair); Q only for the core's own 1024 rows.
    - attention + out-proj for the core's 1024 query rows.
  Host concatenates the 8 disjoint row blocks into the full [4, 2048, 1024].

Layout strategy per core:
  - x chunks are transposed on the tensor engine (via identity matmul) to get
    the contraction dim (D) on partitions for the QKV projections.
  - Q/K are projected in "natural" [l, d] layout, RMS-normed + roped there
    (free-dim reduces), then transposed per head to q^T/k^T [hd, l], packed
    two heads per 128 partitions.
  - S^T = k^T.T @ q^T is computed per (head, 128-row lk chunk) into PSUM
    [128 lk, 1024 lq]; ScalarE applies exp(0.125*S) into SBUF; the AV matmul
    uses [v_h | ones] as the stationary operand so PSUM row 64 accumulates the
    softmax denominator for free.  attn^T is normalized afterwards and used
    directly as the stationary operand of the out projection.
  - Matmuls run in float32r (fp32 bits, FP22 multiplies) which streams at
    bf16 rate for moving free dims >= 256.
"""

import sys

if "/opt/trn_rl_repo" not in sys.path:
    sys.path.insert(0, "/opt/trn_rl_repo")

from contextlib import ExitStack

import numpy as np

import concourse.bass as bass
import concourse.tile as tile
from concourse import mybir, bass_utils
from concourse.masks import make_identity
from concourse.vector_clock import ScopedClock, VectorClock

B, L, D, H = 4, 2048, 1024, 16
HD = D // H          # 64
HHD = HD // 2        # 32
EPS = 1e-6
THETA = 10000.0
N_CORES = 8
LQ = L // 2          # query rows per core
P = 128
NCK = L // P         # 16 l-chunks for K/V
NCQ = LQ // P        # 8 l-chunks for Q
NDC = D // P         # 8 contraction chunks
F32 = mybir.dt.float32
FR = mybir.dt.float32r
AF = mybir.ActivationFunctionType


def _patch_tile_drain():
    """This container's walrus rejects >1 sem wait per instruction.
    Tile's kernel-tail drain waits on every active proc at once; split those
    waits across single-wait NOPs on SP so the drain itself needs none."""
    if getattr(tile.TileContext, "_drain_split_patched", False):
        return

    def _patched(self, tick_clock, wait_clock):
        vc = tick_clock.global_clock
        n = len(vc)
        cur = VectorClock([0] * n)
        for proc in range(n):
            t = vc[proc]
            if t > 0:
                nop = self.nc.sync.nop(hint=f"drainsplit_{proc}", nofuse=True)
                req = VectorClock([0] * n)
                req.require_at_least(proc, t)
                wait_clock.add_sem_waits(
                    nop.ins, ScopedClock({None: req}), ScopedClock({None: cur.copy()})
                )
                cur.require_at_least(proc, t)
        drain_inst = self.nc.sync.drain()
        wait_clock.add_sem_waits(
            drain_inst.ins, ScopedClock({None: vc}), ScopedClock({None: cur})
        )
        self.nc.all_engine_barrier()
        popped = self.nc._tile_sem_poison_stack.pop()
        assert popped is self._sem_poison
        self.nc.clear_and_free_semaphores(list(self.sems.allocated().values()))
        self.nc.all_engine_barrier()

    tile.TileContext._drain_and_barrier = _patched
    tile.TileContext._drain_split_patched = True


def _split_waits(nc, maxw=1):
    """This walrus build allows at most one sync wait per instruction.
    After Tile has assigned semaphores, hoist excess waits onto NOPs
    inserted just before the over-subscribed instruction (same engine,
    same block) — semantically identical, since all waits must clear
    before the instruction executes either way."""
    nid = 0
    for fn in nc.m.functions:
        for bb in fn.blocks:
            insts = list(bb.instructions)
            new = []
            changed = False
            for inst in insts:
                si = inst.sync_info
                if si is not None and si.on_wait is not None and len(si.on_wait) > maxw:
                    waits = list(si.on_wait)
                    extra, keep = waits[:-maxw], waits[-maxw:]
                    for i in range(0, len(extra), maxw):
                        nid += 1
                        new.append(mybir.InstNoOp(
                            name=f"I-wsplit-{nid}", engine=inst.engine,
                            sync_info=mybir.SyncInfo(
                                on_wait=extra[i : i + maxw], on_update=[]),
                        ))
                    inst.sync_info = mybir.SyncInfo(
                        on_wait=keep, on_update=list(si.on_update))
                    changed = True
                new.append(inst)
            if changed:
                bb.instructions = new


def _bcast_free(ap, repeat, at):
    """Insert a step-0 free dim of size `repeat` at free-dim position `at`
    (0 = right after the partition dim)."""
    new = ap.copy()
    new.ap = new.ap[: 1 + at] + [[0, repeat]] + new.ap[1 + at :]
    return new


def _fr(ap):
    if ap.dtype == FR:
        return ap
    return ap.bitcast(FR)


def _build_program(use_bq, use_bk, use_bv, use_bout, use_qnw, use_knw):
    nc = bass.Bass("TRN2", target_bir_lowering=False, debug=False,
                   num_devices=N_CORES)

    xb = nc.dram_tensor("xb", [L, D], F32, kind="ExternalInput").ap()
    xq = nc.dram_tensor("xq", [LQ, D], F32, kind="ExternalInput").ap()
    wq = nc.dram_tensor("wq", [D, D], F32, kind="ExternalInput").ap()
    wk = nc.dram_tensor("wk", [D, D], F32, kind="ExternalInput").ap()
    wv = nc.dram_tensor("wv", [D, D], F32, kind="ExternalInput").ap()
    wout = nc.dram_tensor("wout", [D, D], F32, kind="ExternalInput").ap()
    cosk = nc.dram_tensor("cosk", [L, HHD], F32, kind="ExternalInput").ap()
    sink = nc.dram_tensor("sink", [L, HHD], F32, kind="ExternalInput").ap()
    cosq = nc.dram_tensor("cosq", [LQ, HHD], F32, kind="ExternalInput").ap()
    sinq = nc.dram_tensor("sinq", [LQ, HHD], F32, kind="ExternalInput").ap()
    bq = bk = bv = bo = qnw = knw = None
    if use_bq:
        bq = nc.dram_tensor("bq", [1, D], F32, kind="ExternalInput").ap()
    if use_bk:
        bk = nc.dram_tensor("bk", [1, D], F32, kind="ExternalInput").ap()
    if use_bv:
        bv = nc.dram_tensor("bv", [1, D], F32, kind="ExternalInput").ap()
    if use_bout:
        bo = nc.dram_tensor("bout", [1, D], F32, kind="ExternalInput").ap()
    if use_qnw:
        qnw = nc.dram_tensor("qnw", [1, HD], F32, kind="ExternalInput").ap()
    if use_knw:
        knw = nc.dram_tensor("knw", [1, HD], F32, kind="ExternalInput").ap()
    out = nc.dram_tensor("out", [LQ, D], F32, kind="ExternalOutput").ap()

    with tile.TileContext(nc) as tc, ExitStack() as ctx:
        pers = ctx.enter_context(tc.tile_pool(name="pers", bufs=1))
        dpool = ctx.enter_context(tc.tile_pool(name="dram", bufs=1, space="DRAM"))

        ident = pers.tile([P, P], F32, tag="ident")
        make_identity(nc, ident)

        cosk_sb = pers.tile([P, NCK, HHD], F32, tag="cosk")
        sink_sb = pers.tile([P, NCK, HHD], F32, tag="sink")
        cosq_sb = pers.tile([P, NCQ, HHD], F32, tag="cosq")
        sinq_sb = pers.tile([P, NCQ, HHD], F32, tag="sinq")
        nc.sync.dma_start(out=cosk_sb, in_=cosk.rearrange("(c p) f -> p c f", p=P))
        nc.sync.dma_start(out=sink_sb, in_=sink.rearrange("(c p) f -> p c f", p=P))
        nc.sync.dma_start(out=cosq_sb, in_=cosq.rearrange("(c p) f -> p c f", p=P))
        nc.sync.dma_start(out=sinq_sb, in_=sinq.rearrange("(c p) f -> p c f", p=P))

        # packed transposed activations: pair tile p holds head 2p in
        # partitions 0:64 and head 2p+1 in partitions 64:128.
        kT = pers.tile([P, H // 2, L], FR, tag="kT")
        qT = pers.tile([P, H // 2, LQ], FR, tag="qT")

        ones16 = pers.tile([P, H], F32, tag="ones16")
        nc.vector.memset(ones16, 1.0)
        eps_sb = pers.tile([P, 1], F32, tag="eps")
        nc.vector.memset(eps_sb, EPS)
        ones1 = None
        if use_bq or use_bk or use_bv:
            ones1f = pers.tile([1, P], F32, tag="ones1f")
            nc.vector.memset(ones1f, 1.0)
            ones1 = pers.tile([1, P], FR, tag="ones1")
            nc.vector.tensor_copy(ones1, ones1f)
        qnw_b = knw_b = bout_b = None
        if use_qnw:
            qnw_b = pers.tile([P, HD], F32, tag="qnw_b")
            nc.sync.dma_start(
                out=qnw_b,
                in_=bass.AP(tensor=qnw.tensor, offset=qnw.offset,
                            ap=[[0, P], [1, HD]]),
            )
        if use_knw:
            knw_b = pers.tile([P, HD], F32, tag="knw_b")
            nc.sync.dma_start(
                out=knw_b,
                in_=bass.AP(tensor=knw.tensor, offset=knw.offset,
                            ap=[[0, P], [1, HD]]),
            )
        if use_bout:
            bout_b = pers.tile([P, D], F32, tag="bout_b")
            nc.sync.dma_start(
                out=bout_b,
                in_=bass.AP(tensor=bo.tensor, offset=bo.offset,
                            ap=[[0, P], [1, D]]),
            )

        # DRAM staging: v with a ones column appended (AV stationary operand),
        # and per-head softmax denominators for the broadcast bounce.
        vstage = dpool.tile([L, D], F32, tag="vstage")
        onescol = dpool.tile([L, 1], F32, tag="onescol")
        invstage = dpool.tile([H, LQ], F32, tag="invstage")
        nc.sync.dma_start(out=onescol, in_=ones16)

        def load_w(pool, w_dram, tag):
            w_sb = pool.tile([P, NDC, D], FR, tag=tag)
            nc.sync.dma_start(
                out=w_sb,
                in_=w_dram.rearrange("(j p) n -> p j n", p=P).bitcast(FR),
            )
            return w_sb

        def load_bias(pool, b_dram, tag):
            b_sb = pool.tile([1, D], FR, tag=tag)
            nc.sync.dma_start(out=b_sb, in_=b_dram.bitcast(FR))
            return b_sb

        def proj_chunk(x_dram, ci, w_sb, b_sb, xpool, tppool, pspool):
            """Project one 128-row chunk: returns PSUM [128, D] = x_chunk @ W (+b)."""
            xc = xpool.tile([P, D], F32, tag="xc")
            nc.sync.dma_start(out=xc, in_=x_dram[ci * P : (ci + 1) * P, :])
            xt = xpool.tile([P, NDC, P], FR, tag="xt")
            for half in range(2):
                tp = tppool.tile([P, NDC * P // 2], F32, tag="tp")
                for jj in range(NDC // 2):
                    j = half * (NDC // 2) + jj
                    nc.tensor.transpose(
                        tp[:, jj * P : (jj + 1) * P], xc[:, j * P : (j + 1) * P],
                        ident,
                    )
                dst = xt[:, half * (NDC // 2) : (half + 1) * (NDC // 2), :]
                tps = tp.rearrange("p (j q) -> p j q", j=NDC // 2)
                if half == 0:
                    nc.scalar.copy(dst, tps)
                else:
                    nc.vector.tensor_copy(dst, tps)
            ps = pspool.tile([P, D], F32, tag="ps")
            for n0 in range(0, D, 512):
                for j in range(NDC):
                    nc.tensor.matmul(
                        ps[:, n0 : n0 + 512],
                        _fr(xt[:, j, :]),
                        _fr(w_sb[:, j, n0 : n0 + 512]),
                        start=(j == 0),
                        stop=(j == NDC - 1 and b_sb is None),
                    )
                if b_sb is not None:
                    nc.tensor.matmul(
                        ps[:, n0 : n0 + 512],
                        _fr(ones1),
                        _fr(b_sb[:, n0 : n0 + 512]),
                        start=False,
                        stop=True,
                    )
            return ps

        def norm_rope(ps, cos_ap, sin_ap, nw_b, stg):
            """RMSNorm over head_dim + rotary embed, from PSUM [128, D] in
            natural layout; returns SBUF tile [128, H, HD]."""
            sq = stg.tile([P, D], F32, tag="big")
            nc.scalar.activation(sq, ps, AF.Square)
            ss = stg.tile([P, H], F32, tag="ss")
            nc.vector.tensor_reduce(
                ss, sq.rearrange("p (h d) -> p h d", h=H),
                axis=mybir.AxisListType.X, op=mybir.AluOpType.add,
            )
            inv = stg.tile([P, H], F32, tag="inv")
            nc.scalar.activation(inv, ss, AF.Sqrt, scale=1.0 / HD, bias=eps_sb)
            nc.vector.reciprocal(inv, inv)
            ps3 = ps.rearrange("p (h d) -> p h d", h=H)
            kn = stg.tile([P, H, HD], F32, tag="kn")
            nc.vector.tensor_mul(kn, ps3, _bcast_free(inv, HD, 1))
            if nw_b is not None:
                nc.vector.tensor_mul(kn, kn, _bcast_free(nw_b, H, 0))
            t1 = kn[:, :, 0:HHD]
            t2 = kn[:, :, HHD:HD]
            cosc = _bcast_free(cos_ap, H, 0)
            sinc = _bcast_free(sin_ap, H, 0)
            ra = stg.tile([P, H, HHD], F32, tag="ra")
            rb = stg.tile([P, H, HHD], F32, tag="rb")
            rc = stg.tile([P, H, HHD], F32, tag="rc")
            rd = stg.tile([P, H, HHD], F32, tag="rd")
            rot = stg.tile([P, H, HD], F32, tag="big")
            nc.gpsimd.tensor_mul(ra, t1, cosc)
            nc.gpsimd.tensor_mul(rb, t2, sinc)
            nc.vector.tensor_sub(rot[:, :, 0:HHD], ra, rb)
            nc.gpsimd.tensor_mul(rc, t1, sinc)
            nc.gpsimd.tensor_mul(rd, t2, cosc)
            nc.vector.tensor_add(rot[:, :, HHD:HD], rc, rd)
            return rot

        def transpose_heads(rot, dstT, ci, tp2pool):
            """Per-head PE transpose of [128, HD] -> [HD, 128], packed into
            dstT[(h%2)*64:(h%2)*64+64, h//2, ci*128:ci*128+128]."""
            # group = (parity e, pair-half g): 4 heads h = 2*(4g+pi)+e; one
            # batched copy per group into dstT[e*64:(e+1)*64, 4g:4g+4, ci].
            for e in range(2):
                for g in range(2):
                    t2 = tp2pool.tile([HD, 4, P], F32, tag="t2")
                    for pp in range(4):
                        h = 2 * (4 * g + pp) + e
                        nc.tensor.transpose(t2[:, pp, :], rot[:, h, :], ident)
                    dst = dstT[e * HD : (e + 1) * HD, 4 * g : 4 * g + 4,
                               ci * P : (ci + 1) * P]
                    if e == 0:
                        nc.vector.tensor_copy(dst, t2)
                    else:
                        nc.scalar.copy(dst, t2)

        # ---- Phase K: project+norm+rope+transpose K for all 16 chunks ----
        with ExitStack() as ph:
            wpool = ph.enter_context(tc.tile_pool(name="wpk", bufs=1))
            xpool = ph.enter_context(tc.tile_pool(name="xk", bufs=3))
            stg = ph.enter_context(tc.tile_pool(name="stgk", bufs=2))
            pspool = ph.enter_context(tc.tile_pool(name="psk", bufs=2, space="PSUM"))
            tppool = ph.enter_context(tc.tile_pool(name="tpk", bufs=2, space="PSUM"))
            tp2pool = ph.enter_context(tc.tile_pool(name="tp2k", bufs=2, space="PSUM"))
            w_sb = load_w(wpool, wk, "wk_sb")
            b_sb = load_bias(wpool, bk, "bk_sb") if use_bk else None
            for ci in range(NCK):
                ps = proj_chunk(xb, ci, w_sb, b_sb, xpool, tppool, pspool)
                rot = norm_rope(ps, cosk_sb[:, ci, :], sink_sb[:, ci, :], knw_b, stg)
                transpose_heads(rot, kT, ci, tp2pool)

        # ---- Phase Q: same for the core's own 8 chunks ----
        with ExitStack() as ph:
            wpool = ph.enter_context(tc.tile_pool(name="wpq", bufs=1))
            xpool = ph.enter_context(tc.tile_pool(name="xq", bufs=3))
            stg = ph.enter_context(tc.tile_pool(name="stgq", bufs=2))
            pspool = ph.enter_context(tc.tile_pool(name="psq", bufs=2, space="PSUM"))
            tppool = ph.enter_context(tc.tile_pool(name="tpq", bufs=2, space="PSUM"))
            tp2pool = ph.enter_context(tc.tile_pool(name="tp2q", bufs=2, space="PSUM"))
            w_sb = load_w(wpool, wq, "wq_sb")
            b_sb = load_bias(wpool, bq, "bq_sb") if use_bq else None
            for ci in range(NCQ):
                ps = proj_chunk(xq, ci, w_sb, b_sb, xpool, tppool, pspool)
                rot = norm_rope(ps, cosq_sb[:, ci, :], sinq_sb[:, ci, :], qnw_b, stg)
                transpose_heads(rot, qT, ci, tp2pool)

        # ---- Phase V: project V for all 16 chunks, stage to DRAM ----
        with ExitStack() as ph:
            wpool = ph.enter_context(tc.tile_pool(name="wpv", bufs=1))
            xpool = ph.enter_context(tc.tile_pool(name="xv", bufs=3))
            stg = ph.enter_context(tc.tile_pool(name="stgv", bufs=2))
            pspool = ph.enter_context(tc.tile_pool(name="psv", bufs=2, space="PSUM"))
            tppool = ph.enter_context(tc.tile_pool(name="tpv", bufs=2, space="PSUM"))
            w_sb = load_w(wpool, wv, "wv_sb")
            b_sb = load_bias(wpool, bv, "bv_sb") if use_bv else None
            for ci in range(NCK):
                ps = proj_chunk(xb, ci, w_sb, b_sb, xpool, tppool, pspool)
                vsb = stg.tile([P, D], F32, tag="vsb")
                nc.scalar.copy(vsb[:, 0 : D // 2], ps[:, 0 : D // 2])
                nc.vector.tensor_copy(vsb[:, D // 2 : D], ps[:, D // 2 : D])
                nc.sync.dma_start(
                    out=vstage[ci * P : (ci + 1) * P, :], in_=vsb
                )

        # ---- Phase attention + out projection ----
        with ExitStack() as ph:
            wpool = ph.enter_context(tc.tile_pool(name="wpo", bufs=1))
            wout_sb = load_w(wpool, wout, "wout_sb")
            attnT = wpool.tile([P, H // 2, LQ], FR, tag="attnT")
            with ExitStack() as ph2:
                ppool = ph2.enter_context(tc.tile_pool(name="pt", bufs=2))
                vcpool = ph2.enter_context(tc.tile_pool(name="vc", bufs=3))
                invpool = ph2.enter_context(tc.tile_pool(name="invp", bufs=2))
                upool = ph2.enter_context(tc.tile_pool(name="ups", bufs=2, space="PSUM"))
                spool = ph2.enter_context(tc.tile_pool(name="sps", bufs=2, space="PSUM"))
                def s_chunk(h, ci):
                    pi, po = h // 2, (h % 2) * HD
                    sT = spool.tile([P, LQ], F32, tag="sT")
                    kslice = kT[po : po + HD, pi, ci * P : (ci + 1) * P]
                    for n0 in range(0, LQ, 512):
                        nc.tensor.matmul(
                            sT[:, n0 : n0 + 512],
                            kslice,
                            qT[po : po + HD, pi, n0 : n0 + 512],
                            start=True,
                            stop=True,
                        )
                    return sT

                def av_chunk(h, ci, sT, U):
                    Pt = ppool.tile([P, LQ], FR, tag="Pt")
                    nc.scalar.activation(Pt, sT, AF.Exp, scale=HD ** -0.5)
                    vc = vcpool.tile([P, HD + 1], FR, tag="vc")
                    nc.gpsimd.dma_start(
                        out=vc[:, 0:HD],
                        in_=vstage[ci * P : (ci + 1) * P,
                                   (h * HD) : (h + 1) * HD].bitcast(FR),
                    )
                    nc.gpsimd.dma_start(
                        out=vc[:, HD : HD + 1],
                        in_=onescol[ci * P : (ci + 1) * P, :].bitcast(FR),
                    )
                    for n0 in range(0, LQ, 512):
                        nc.tensor.matmul(
                            U[:, n0 : n0 + 512],
                            vc,
                            Pt[:, n0 : n0 + 512],
                            start=(ci == 0),
                            stop=(ci == NCK - 1),
                        )

                for h in range(H):
                    pi, po = h // 2, (h % 2) * HD
                    U = upool.tile([HD + 1, LQ], F32, tag="U")
                    sT_prev = s_chunk(h, 0)
                    for ci in range(1, NCK):
                        sT_cur = s_chunk(h, ci)
                        av_chunk(h, ci - 1, sT_prev, U)
                        sT_prev = sT_cur
                    av_chunk(h, NCK - 1, sT_prev, U)
                    nc.vector.tensor_copy(attnT[po : po + HD, pi, :], U[0:HD, :])
                    inv = invpool.tile([1, LQ], F32, tag="inv")
                    nc.vector.reciprocal(inv, U[HD : HD + 1, :])
                    nc.sync.dma_start(out=invstage[h, :], in_=inv)

            with ExitStack() as ph2:
                bcpool = ph2.enter_context(tc.tile_pool(name="bcp", bufs=2))
                opool = ph2.enter_context(tc.tile_pool(name="ops", bufs=2, space="PSUM"))
                obpool = ph2.enter_context(tc.tile_pool(name="obp", bufs=2))
                for pi in range(H // 2):
                    bc = bcpool.tile([P, LQ], F32, tag="bc")
                    for hh in range(2):
                        src = invstage[2 * pi + hh, :]
                        nc.sync.dma_start(
                            out=bc[hh * HD : (hh + 1) * HD, :],
                            in_=bass.AP(tensor=src.tensor, offset=src.offset,
                                        ap=[[0, HD], [1, LQ]]),
                        )
                    nc.vector.tensor_mul(attnT[:, pi, :], attnT[:, pi, :], bc)
                for cj in range(NCQ):
                    pso = opool.tile([P, D], F32, tag="pso")
                    for n0 in range(0, D, 512):
                        for j in range(NDC):
                            nc.tensor.matmul(
                                pso[:, n0 : n0 + 512],
                                _fr(attnT[:, j, cj * P : (cj + 1) * P]),
                                _fr(wout_sb[:, j, n0 : n0 + 512]),
                                start=(j == 0),
                                stop=(j == NDC - 1),
                            )
                    ob = obpool.tile([P, D], F32, tag="ob")
                    if use_bout:
                        nc.vector.tensor_add(ob, pso, bout_b)
                    else:
                        nc.vector.tensor_copy(ob, pso)
                    nc.sync.dma_start(out=out[cj * P : (cj + 1) * P, :], in_=ob)

    return nc


_PROGRAM_CACHE = {}


def _get_program(flags):
    if flags not in _PROGRAM_CACHE:
        _patch_tile_drain()
        _PROGRAM_CACHE[flags] = _build_program(*flags)
    return _PROGRAM_CACHE[flags]


def _rope_tables():
    pos = np.arange(L, dtype=np.float32)
    inv_freq = (1.0 / (THETA ** (np.arange(0, HD, 2, dtype=np.float32) / HD))).astype(
        np.float32
    )
    ang = pos[:, None] * inv_freq[None, :]
    return np.cos(ang).astype(np.float32), np.sin(ang).astype(np.float32)


def _make_in_maps(x, Wqkv, bqkv, qn_w, kn_w, Wout, bout, flags):
    use_bq, use_bk, use_bv, use_bout, use_qnw, use_knw = flags
    cos, sin = _rope_tables()
    wq = np.ascontiguousarray(Wqkv[:, 0:D])
    wk = np.ascontiguousarray(Wqkv[:, D : 2 * D])
    wv = np.ascontiguousarray(Wqkv[:, 2 * D : 3 * D])
    in_maps = []
    for c in range(N_CORES):
        b, half = c // 2, c % 2
        m = {
            "xb": np.ascontiguousarray(x[b]),
            "xq": np.ascontiguousarray(x[b, half * LQ : (half + 1) * LQ, :]),
            "wq": wq,
            "wk": wk,
            "wv": wv,
            "wout": np.ascontiguousarray(Wout),
            "cosk": cos,
            "sink": sin,
            "cosq": np.ascontiguousarray(cos[half * LQ : (half + 1) * LQ]),
            "sinq": np.ascontiguousarray(sin[half * LQ : (half + 1) * LQ]),
        }
        if use_bq:
            m["bq"] = np.ascontiguousarray(bqkv[0:D]).reshape(1, D)
        if use_bk:
            m["bk"] = np.ascontiguousarray(bqkv[D : 2 * D]).reshape(1, D)
        if use_bv:
            m["bv"] = np.ascontiguousarray(bqkv[2 * D : 3 * D]).reshape(1, D)
        if use_bout:
            m["bout"] = np.ascontiguousarray(bout).reshape(1, D)
        if use_qnw:
            m["qnw"] = np.ascontiguousarray(qn_w).reshape(1, HD)
        if use_knw:
            m["knw"] = np.ascontiguousarray(kn_w).reshape(1, HD)
        in_maps.append(m)
    return in_maps


def _flags_for(bqkv, qn_w, kn_w, bout):
    return (
        bool(np.any(bqkv[0:D])),
        bool(np.any(bqkv[D : 2 * D])),
        bool(np.any(bqkv[2 * D : 3 * D])),
        bool(np.any(bout)),
        bool(np.any(qn_w != 1.0)),
        bool(np.any(kn_w != 1.0)),
    )


def _assemble(results):
    out = np.empty((B, L, D), dtype=np.float32)
    for c in range(N_CORES):
        b, half = c // 2, c % 2
        out[b, half * LQ : (half + 1) * LQ, :] = results[c]["out"]
    return out


def kernel(x, Wqkv, bqkv, qn_w, kn_w, Wout, bout, _trace=False):
    x = np.asarray(x, dtype=np.float32)
    Wqkv = np.asarray(Wqkv, dtype=np.float32)
    bqkv = np.asarray(bqkv, dtype=np.float32)
    qn_w = np.asarray(qn_w, dtype=np.float32)
    kn_w = np.asarray(kn_w, dtype=np.float32)
    Wout = np.asarray(Wout, dtype=np.float32)
    bout = np.asarray(bout, dtype=np.float32)

    flags = _flags_for(bqkv, qn_w, kn_w, bout)
    nc = _get_program(flags)
    if not getattr(nc, "_waits_split", False):
        _split_waits(nc)
        nc._waits_split = True
    in_maps = _make_in_maps(x, Wqkv, bqkv, qn_w, kn_w, Wout, bout, flags)
    res = bass_utils.run_bass_kernel_spmd(
        nc, in_maps, core_ids=list(range(N_CORES))
    )
    out = _assemble(res.results)
    if _trace:
        return out, res
    return out


# revision 15
# speedup vs baseline: 18.9870x; 2.2264x over previous
"""DiT attention block (QKV proj + QK-RMSNorm + RoPE + softmax attention + out proj)
as a Bass/Tile kernel for 8 Trainium2 NeuronCores.

Sharding (zero cross-core communication):
  core c -> batch b = c//2, sequence half = c%2.
  Each core computes output rows [half*1024, half*1024+1024) of batch b:
    - K, V are computed for the full 2048-row sequence of batch b (duplicated
      within a core pair); Q only for the core's own 1024 rows.
    - attention + out-proj for the core's 1024 query rows.
  Host concatenates the 8 disjoint row blocks into the full [4, 2048, 1024].

Layout strategy per core:
  - x chunks are transposed on the tensor engine (via identity matmul) to get
    the contraction dim (D) on partitions for the QKV projections.
  - Q/K are projected in "natural" [l, d] layout, RMS-normed + roped there
    (free-dim reduces), then transposed per head to q^T/k^T [hd, l], packed
    two heads per 128 partitions.
  - S^T = k^T.T @ q^T is computed per (head, 128-row lk chunk) into PSUM
    [128 lk, 1024 lq]; ScalarE applies exp(0.125*S) into SBUF; the AV matmul
    uses [v_h | ones] as the stationary operand so PSUM row 64 accumulates the
    softmax denominator for free.  attn^T is normalized afterwards and used
    directly as the stationary operand of the out projection.
  - Matmuls run in float32r (fp32 bits, FP22 multiplies) which streams at
    bf16 rate for moving free dims >= 256.
"""

import sys

if "/opt/trn_rl_repo" not in sys.path:
    sys.path.insert(0, "/opt/trn_rl_repo")

from contextlib import ExitStack

import numpy as np

import concourse.bass as bass
import concourse.tile as tile
from concourse import mybir, bass_utils
from concourse.masks import make_identity
from concourse.vector_clock import ScopedClock, VectorClock

B, L, D, H = 4, 2048, 1024, 16
HD = D // H          # 64
HHD = HD // 2        # 32
EPS = 1e-6
THETA = 10000.0
N_CORES = 8
LQ = L // 2          # query rows per core
P = 128
NCK = L // P         # 16 l-chunks for K/V
NCQ = LQ // P        # 8 l-chunks for Q
NDC = D // P         # 8 contraction chunks
F32 = mybir.dt.float32
FR = mybir.dt.float32r
AF = mybir.ActivationFunctionType


def _patch_tile_drain():
    """This container's walrus rejects >1 sem wait per instruction.
    Tile's kernel-tail drain waits on every active proc at once; split those
    waits across single-wait NOPs on SP so the drain itself needs none."""
    if getattr(tile.TileContext, "_drain_split_patched", False):
        return

    def _patched(self, tick_clock, wait_clock):
        vc = tick_clock.global_clock
        n = len(vc)
        cur = VectorClock([0] * n)
        for proc in range(n):
            t = vc[proc]
            if t > 0:
                nop = self.nc.sync.nop(hint=f"drainsplit_{proc}", nofuse=True)
                req = VectorClock([0] * n)
                req.require_at_least(proc, t)
                wait_clock.add_sem_waits(
                    nop.ins, ScopedClock({None: req}), ScopedClock({None: cur.copy()})
                )
                cur.require_at_least(proc, t)
        drain_inst = self.nc.sync.drain()
        wait_clock.add_sem_waits(
            drain_inst.ins, ScopedClock({None: vc}), ScopedClock({None: cur})
        )
        self.nc.all_engine_barrier()
        popped = self.nc._tile_sem_poison_stack.pop()
        assert popped is self._sem_poison
        self.nc.clear_and_free_semaphores(list(self.sems.allocated().values()))
        self.nc.all_engine_barrier()

    tile.TileContext._drain_and_barrier = _patched
    tile.TileContext._drain_split_patched = True


def _split_waits(nc, maxw=1):
    """This walrus build allows at most one sync wait per instruction.
    After Tile has assigned semaphores, hoist excess waits onto NOPs
    inserted just before the over-subscribed instruction (same engine,
    same block) — semantically identical, since all waits must clear
    before the instruction executes either way."""
    nid = 0
    for fn in nc.m.functions:
        for bb in fn.blocks:
            insts = list(bb.instructions)
            new = []
            changed = False
            for inst in insts:
                si = inst.sync_info
                if si is not None and si.on_wait is not None and len(si.on_wait) > maxw:
                    waits = list(si.on_wait)
                    extra, keep = waits[:-maxw], waits[-maxw:]
                    for i in range(0, len(extra), maxw):
                        nid += 1
                        new.append(mybir.InstNoOp(
                            name=f"I-wsplit-{nid}", engine=inst.engine,
                            sync_info=mybir.SyncInfo(
                                on_wait=extra[i : i + maxw], on_update=[]),
                        ))
                    inst.sync_info = mybir.SyncInfo(
                        on_wait=keep, on_update=list(si.on_update))
                    changed = True
                new.append(inst)
            if changed:
                bb.instructions = new


def _bcast_free(ap, repeat, at):
    """Insert a step-0 free dim of size `repeat` at free-dim position `at`
    (0 = right after the partition dim)."""
    new = ap.copy()
    new.ap = new.ap[: 1 + at] + [[0, repeat]] + new.ap[1 + at :]
    return new


def _fr(ap):
    if ap.dtype == FR:
        return ap
    return ap.bitcast(FR)


def _build_program(use_bq, use_bk, use_bv, use_bout, use_qnw, use_knw):
    nc = bass.Bass("TRN2", target_bir_lowering=False, debug=False,
                   num_devices=N_CORES)

    xb = nc.dram_tensor("xb", [L, D], F32, kind="ExternalInput").ap()
    xq = nc.dram_tensor("xq", [LQ, D], F32, kind="ExternalInput").ap()
    wq = nc.dram_tensor("wq", [D, D], F32, kind="ExternalInput").ap()
    wk = nc.dram_tensor("wk", [D, D], F32, kind="ExternalInput").ap()
    wv = nc.dram_tensor("wv", [D, D], F32, kind="ExternalInput").ap()
    wout = nc.dram_tensor("wout", [D, D], F32, kind="ExternalInput").ap()
    cosk = nc.dram_tensor("cosk", [L, HHD], F32, kind="ExternalInput").ap()
    sink = nc.dram_tensor("sink", [L, HHD], F32, kind="ExternalInput").ap()
    cosq = nc.dram_tensor("cosq", [LQ, HHD], F32, kind="ExternalInput").ap()
    sinq = nc.dram_tensor("sinq", [LQ, HHD], F32, kind="ExternalInput").ap()
    bq = bk = bv = bo = qnw = knw = None
    if use_bq:
        bq = nc.dram_tensor("bq", [1, D], F32, kind="ExternalInput").ap()
    if use_bk:
        bk = nc.dram_tensor("bk", [1, D], F32, kind="ExternalInput").ap()
    if use_bv:
        bv = nc.dram_tensor("bv", [1, D], F32, kind="ExternalInput").ap()
    if use_bout:
        bo = nc.dram_tensor("bout", [1, D], F32, kind="ExternalInput").ap()
    if use_qnw:
        qnw = nc.dram_tensor("qnw", [1, HD], F32, kind="ExternalInput").ap()
    if use_knw:
        knw = nc.dram_tensor("knw", [1, HD], F32, kind="ExternalInput").ap()
    out = nc.dram_tensor("out", [LQ, D], F32, kind="ExternalOutput").ap()

    with tile.TileContext(nc) as tc, ExitStack() as ctx:
        pers = ctx.enter_context(tc.tile_pool(name="pers", bufs=1))
        dpool = ctx.enter_context(tc.tile_pool(name="dram", bufs=1, space="DRAM"))

        ident = pers.tile([P, P], F32, tag="ident")
        make_identity(nc, ident)

        cosk_sb = pers.tile([P, NCK, HHD], F32, tag="cosk")
        sink_sb = pers.tile([P, NCK, HHD], F32, tag="sink")
        cosq_sb = pers.tile([P, NCQ, HHD], F32, tag="cosq")
        sinq_sb = pers.tile([P, NCQ, HHD], F32, tag="sinq")
        nc.sync.dma_start(out=cosk_sb, in_=cosk.rearrange("(c p) f -> p c f", p=P))
        nc.sync.dma_start(out=sink_sb, in_=sink.rearrange("(c p) f -> p c f", p=P))
        nc.sync.dma_start(out=cosq_sb, in_=cosq.rearrange("(c p) f -> p c f", p=P))
        nc.sync.dma_start(out=sinq_sb, in_=sinq.rearrange("(c p) f -> p c f", p=P))

        # packed transposed activations: pair tile p holds head 2p in
        # partitions 0:64 and head 2p+1 in partitions 64:128.
        kT = pers.tile([P, H // 2, L], FR, tag="kT")
        qT = pers.tile([P, H // 2, LQ], FR, tag="qT")

        ones16 = pers.tile([P, H], F32, tag="ones16")
        nc.vector.memset(ones16, 1.0)
        eps_sb = pers.tile([P, 1], F32, tag="eps")
        nc.vector.memset(eps_sb, EPS)
        ones1 = None
        if use_bq or use_bk or use_bv:
            ones1f = pers.tile([1, P], F32, tag="ones1f")
            nc.vector.memset(ones1f, 1.0)
            ones1 = pers.tile([1, P], FR, tag="ones1")
            nc.vector.tensor_copy(ones1, ones1f)
        qnw_b = knw_b = bout_b = None
        if use_qnw:
            qnw_b = pers.tile([P, HD], F32, tag="qnw_b")
            nc.sync.dma_start(
                out=qnw_b,
                in_=bass.AP(tensor=qnw.tensor, offset=qnw.offset,
                            ap=[[0, P], [1, HD]]),
            )
        if use_knw:
            knw_b = pers.tile([P, HD], F32, tag="knw_b")
            nc.sync.dma_start(
                out=knw_b,
                in_=bass.AP(tensor=knw.tensor, offset=knw.offset,
                            ap=[[0, P], [1, HD]]),
            )
        if use_bout:
            bout_b = pers.tile([P, D], F32, tag="bout_b")
            nc.sync.dma_start(
                out=bout_b,
                in_=bass.AP(tensor=bo.tensor, offset=bo.offset,
                            ap=[[0, P], [1, D]]),
            )

        # DRAM staging: v with a ones column appended (AV stationary operand),
        # and per-head softmax denominators for the broadcast bounce.
        vstage = dpool.tile([L, D], F32, tag="vstage")
        onescol = dpool.tile([L, 1], F32, tag="onescol")
        invstage = dpool.tile([H, LQ], F32, tag="invstage")
        nc.sync.dma_start(out=onescol, in_=ones16)

        def load_w(pool, w_dram, tag):
            w_sb = pool.tile([P, NDC, D], FR, tag=tag)
            nc.sync.dma_start(
                out=w_sb,
                in_=w_dram.rearrange("(j p) n -> p j n", p=P).bitcast(FR),
            )
            return w_sb

        def load_bias(pool, b_dram, tag):
            b_sb = pool.tile([1, D], FR, tag=tag)
            nc.sync.dma_start(out=b_sb, in_=b_dram.bitcast(FR))
            return b_sb

        def proj_chunk(x_dram, ci, w_sb, b_sb, xpool, tppool, pspool):
            """Project one 128-row chunk: returns PSUM [128, D] = x_chunk @ W (+b)."""
            xc = xpool.tile([P, D], F32, tag="xc")
            nc.sync.dma_start(out=xc, in_=x_dram[ci * P : (ci + 1) * P, :])
            xt = xpool.tile([P, NDC, P], FR, tag="xt")
            for half in range(2):
                tp = tppool.tile([P, NDC * P // 2], F32, tag="tp")
                for jj in range(NDC // 2):
                    j = half * (NDC // 2) + jj
                    nc.tensor.transpose(
                        tp[:, jj * P : (jj + 1) * P], xc[:, j * P : (j + 1) * P],
                        ident,
                    )
                dst = xt[:, half * (NDC // 2) : (half + 1) * (NDC // 2), :]
                tps = tp.rearrange("p (j q) -> p j q", j=NDC // 2)
                if half == 0:
                    nc.scalar.copy(dst, tps)
                else:
                    nc.vector.tensor_copy(dst, tps)
            ps = pspool.tile([P, D], F32, tag="ps")
            for n0 in range(0, D, 512):
                for j in range(NDC):
                    nc.tensor.matmul(
                        ps[:, n0 : n0 + 512],
                        _fr(xt[:, j, :]),
                        _fr(w_sb[:, j, n0 : n0 + 512]),
                        start=(j == 0),
                        stop=(j == NDC - 1 and b_sb is None),
                    )
                if b_sb is not None:
                    nc.tensor.matmul(
                        ps[:, n0 : n0 + 512],
                        _fr(ones1),
                        _fr(b_sb[:, n0 : n0 + 512]),
                        start=False,
                        stop=True,
                    )
            return ps

        def norm_rope(ps, cos_ap, sin_ap, nw_b, stg):
            """RMSNorm over head_dim + rotary embed, from PSUM [128, D] in
            natural layout; returns SBUF tile [128, H, HD]."""
            sq = stg.tile([P, D], F32, tag="big")
            nc.scalar.activation(sq, ps, AF.Square)
            ss = stg.tile([P, H], F32, tag="ss")
            nc.vector.tensor_reduce(
                ss, sq.rearrange("p (h d) -> p h d", h=H),
                axis=mybir.AxisListType.X, op=mybir.AluOpType.add,
            )
            inv = stg.tile([P, H], F32, tag="inv")
            nc.scalar.activation(inv, ss, AF.Sqrt, scale=1.0 / HD, bias=eps_sb)
            nc.vector.reciprocal(inv, inv)
            ps3 = ps.rearrange("p (h d) -> p h d", h=H)
            kn = stg.tile([P, H, HD], F32, tag="kn")
            nc.vector.tensor_mul(kn, ps3, _bcast_free(inv, HD, 1))
            if nw_b is not None:
                nc.vector.tensor_mul(kn, kn, _bcast_free(nw_b, H, 0))
            t1 = kn[:, :, 0:HHD]
            t2 = kn[:, :, HHD:HD]
            cosc = _bcast_free(cos_ap, H, 0)
            sinc = _bcast_free(sin_ap, H, 0)
            ra = stg.tile([P, H, HHD], F32, tag="ra")
            rb = stg.tile([P, H, HHD], F32, tag="rb")
            rc = stg.tile([P, H, HHD], F32, tag="rc")
            rd = stg.tile([P, H, HHD], F32, tag="rd")
            rot = stg.tile([P, H, HD], F32, tag="big")
            nc.gpsimd.tensor_mul(ra, t1, cosc)
            nc.gpsimd.tensor_mul(rb, t2, sinc)
            nc.vector.tensor_sub(rot[:, :, 0:HHD], ra, rb)
            nc.gpsimd.tensor_mul(rc, t1, sinc)
            nc.gpsimd.tensor_mul(rd, t2, cosc)
            nc.vector.tensor_add(rot[:, :, HHD:HD], rc, rd)
            return rot

        def transpose_heads(rot, dstT, ci, tp2pool):
            """Per-head PE transpose of [128, HD] -> [HD, 128], packed into
            dstT[(h%2)*64:(h%2)*64+64, h//2, ci*128:ci*128+128]."""
            # group = (parity e, pair-half g): 4 heads h = 2*(4g+pi)+e; one
            # batched copy per group into dstT[e*64:(e+1)*64, 4g:4g+4, ci].
            for e in range(2):
                for g in range(2):
                    t2 = tp2pool.tile([HD, 4, P], F32, tag="t2")
                    for pp in range(4):
                        h = 2 * (4 * g + pp) + e
                        nc.tensor.transpose(t2[:, pp, :], rot[:, h, :], ident)
                    dst = dstT[e * HD : (e + 1) * HD, 4 * g : 4 * g + 4,
                               ci * P : (ci + 1) * P]
                    if e == 0:
                        nc.vector.tensor_copy(dst, t2)
                    else:
                        nc.scalar.copy(dst, t2)

        # ---- Phase K: project+norm+rope+transpose K for all 16 chunks ----
        with ExitStack() as ph:
            wpool = ph.enter_context(tc.tile_pool(name="wpk", bufs=1))
            xpool = ph.enter_context(tc.tile_pool(name="xk", bufs=3))
            stg = ph.enter_context(tc.tile_pool(name="stgk", bufs=2))
            pspool = ph.enter_context(tc.tile_pool(name="psk", bufs=2, space="PSUM"))
            tppool = ph.enter_context(tc.tile_pool(name="tpk", bufs=2, space="PSUM"))
            tp2pool = ph.enter_context(tc.tile_pool(name="tp2k", bufs=2, space="PSUM"))
            w_sb = load_w(wpool, wk, "wk_sb")
            b_sb = load_bias(wpool, bk, "bk_sb") if use_bk else None
            for ci in range(NCK):
                ps = proj_chunk(xb, ci, w_sb, b_sb, xpool, tppool, pspool)
                rot = norm_rope(ps, cosk_sb[:, ci, :], sink_sb[:, ci, :], knw_b, stg)
                transpose_heads(rot, kT, ci, tp2pool)

        # ---- Phase Q: same for the core's own 8 chunks ----
        with ExitStack() as ph:
            wpool = ph.enter_context(tc.tile_pool(name="wpq", bufs=1))
            xpool = ph.enter_context(tc.tile_pool(name="xq", bufs=3))
            stg = ph.enter_context(tc.tile_pool(name="stgq", bufs=2))
            pspool = ph.enter_context(tc.tile_pool(name="psq", bufs=2, space="PSUM"))
            tppool = ph.enter_context(tc.tile_pool(name="tpq", bufs=2, space="PSUM"))
            tp2pool = ph.enter_context(tc.tile_pool(name="tp2q", bufs=2, space="PSUM"))
            w_sb = load_w(wpool, wq, "wq_sb")
            b_sb = load_bias(wpool, bq, "bq_sb") if use_bq else None
            for ci in range(NCQ):
                ps = proj_chunk(xq, ci, w_sb, b_sb, xpool, tppool, pspool)
                rot = norm_rope(ps, cosq_sb[:, ci, :], sinq_sb[:, ci, :], qnw_b, stg)
                transpose_heads(rot, qT, ci, tp2pool)

        # ---- Phase V: project V for all 16 chunks, stage to DRAM ----
        with ExitStack() as ph:
            wpool = ph.enter_context(tc.tile_pool(name="wpv", bufs=1))
            xpool = ph.enter_context(tc.tile_pool(name="xv", bufs=3))
            stg = ph.enter_context(tc.tile_pool(name="stgv", bufs=2))
            pspool = ph.enter_context(tc.tile_pool(name="psv", bufs=2, space="PSUM"))
            tppool = ph.enter_context(tc.tile_pool(name="tpv", bufs=2, space="PSUM"))
            w_sb = load_w(wpool, wv, "wv_sb")
            b_sb = load_bias(wpool, bv, "bv_sb") if use_bv else None
            for ci in range(NCK):
                ps = proj_chunk(xb, ci, w_sb, b_sb, xpool, tppool, pspool)
                vsb = stg.tile([P, D], F32, tag="vsb")
                nc.scalar.copy(vsb[:, 0 : D // 2], ps[:, 0 : D // 2])
                nc.vector.tensor_copy(vsb[:, D // 2 : D], ps[:, D // 2 : D])
                nc.sync.dma_start(
                    out=vstage[ci * P : (ci + 1) * P, :], in_=vsb
                )

        # ---- Phase attention + out projection ----
        with ExitStack() as ph:
            wpool = ph.enter_context(tc.tile_pool(name="wpo", bufs=1))
            wout_sb = load_w(wpool, wout, "wout_sb")
            attnT = wpool.tile([P, H // 2, LQ], FR, tag="attnT")
            with ExitStack() as ph2:
                ppool = ph2.enter_context(tc.tile_pool(name="pt", bufs=2))
                vcpool = ph2.enter_context(tc.tile_pool(name="vc", bufs=3))
                invpool = ph2.enter_context(tc.tile_pool(name="invp", bufs=2))
                bcpool = ph2.enter_context(tc.tile_pool(name="bcp", bufs=2))
                upool = ph2.enter_context(tc.tile_pool(name="ups", bufs=2, space="PSUM"))
                spool = ph2.enter_context(tc.tile_pool(name="sps", bufs=2, space="PSUM"))
                def s_chunk(h, ci):
                    pi, po = h // 2, (h % 2) * HD
                    sT = spool.tile([P, LQ], F32, tag="sT")
                    kslice = kT[po : po + HD, pi, ci * P : (ci + 1) * P]
                    for n0 in range(0, LQ, 512):
                        nc.tensor.matmul(
                            sT[:, n0 : n0 + 512],
                            kslice,
                            qT[po : po + HD, pi, n0 : n0 + 512],
                            start=True,
                            stop=True,
                        )
                    return sT

                def av_chunk(h, ci, sT, U):
                    Pt = ppool.tile([P, LQ], FR, tag="Pt")
                    nc.scalar.activation(Pt, sT, AF.Exp, scale=HD ** -0.5)
                    vc = vcpool.tile([P, HD + 1], FR, tag="vc")
                    nc.sync.dma_start(
                        out=vc[:, 0:HD],
                        in_=vstage[ci * P : (ci + 1) * P,
                                   (h * HD) : (h + 1) * HD].bitcast(FR),
                    )
                    nc.sync.dma_start(
                        out=vc[:, HD : HD + 1],
                        in_=onescol[ci * P : (ci + 1) * P, :].bitcast(FR),
                    )
                    for n0 in range(0, LQ, 512):
                        nc.tensor.matmul(
                            U[:, n0 : n0 + 512],
                            vc,
                            Pt[:, n0 : n0 + 512],
                            start=(ci == 0),
                            stop=(ci == NCK - 1),
                        )

                for h in range(H):
                    pi, po = h // 2, (h % 2) * HD
                    U = upool.tile([HD + 1, LQ], F32, tag="U")
                    sT_prev = s_chunk(h, 0)
                    for ci in range(1, NCK):
                        sT_cur = s_chunk(h, ci)
                        av_chunk(h, ci - 1, sT_prev, U)
                        sT_prev = sT_cur
                    av_chunk(h, NCK - 1, sT_prev, U)
                    nc.vector.tensor_copy(attnT[po : po + HD, pi, :], U[0:HD, :])
                    inv = invpool.tile([1, LQ], F32, tag="inv")
                    nc.vector.reciprocal(inv, U[HD : HD + 1, :])
                    nc.sync.dma_start(out=invstage[h, :], in_=inv)
                    if h % 2 == 1:
                        # both heads of pair pi evicted: normalize in place now,
                        # overlapped with the next head's ACT-bound attention.
                        bc = bcpool.tile([P, LQ], F32, tag="bc")
                        for hh in range(2):
                            iv = invstage[2 * pi + hh, :]
                            nc.sync.dma_start(
                                out=bc[hh * HD : (hh + 1) * HD, :],
                                in_=bass.AP(tensor=iv.tensor, offset=iv.offset,
                                            ap=[[0, HD], [1, LQ]]),
                            )
                        nc.vector.tensor_mul(attnT[:, pi, :], attnT[:, pi, :], bc)

            with ExitStack() as ph2:
                opool = ph2.enter_context(tc.tile_pool(name="ops", bufs=2, space="PSUM"))
                obpool = ph2.enter_context(tc.tile_pool(name="obp", bufs=2))
                for cj in range(NCQ):
                    pso = opool.tile([P, D], F32, tag="pso")
                    for n0 in range(0, D, 512):
                        for j in range(NDC):
                            nc.tensor.matmul(
                                pso[:, n0 : n0 + 512],
                                _fr(attnT[:, j, cj * P : (cj + 1) * P]),
                                _fr(wout_sb[:, j, n0 : n0 + 512]),
                                start=(j == 0),
                                stop=(j == NDC - 1),
                            )
                    ob = obpool.tile([P, D], F32, tag="ob")
                    if use_bout:
                        nc.vector.tensor_add(ob, pso, bout_b)
                    else:
                        nc.vector.tensor_copy(ob, pso)
                    nc.sync.dma_start(out=out[cj * P : (cj + 1) * P, :], in_=ob)

    return nc


_PROGRAM_CACHE = {}


def _get_program(flags):
    if flags not in _PROGRAM_CACHE:
        _patch_tile_drain()
        _PROGRAM_CACHE[flags] = _build_program(*flags)
    return _PROGRAM_CACHE[flags]


def _rope_tables():
    pos = np.arange(L, dtype=np.float32)
    inv_freq = (1.0 / (THETA ** (np.arange(0, HD, 2, dtype=np.float32) / HD))).astype(
        np.float32
    )
    ang = pos[:, None] * inv_freq[None, :]
    return np.cos(ang).astype(np.float32), np.sin(ang).astype(np.float32)


def _make_in_maps(x, Wqkv, bqkv, qn_w, kn_w, Wout, bout, flags):
    use_bq, use_bk, use_bv, use_bout, use_qnw, use_knw = flags
    cos, sin = _rope_tables()
    wq = np.ascontiguousarray(Wqkv[:, 0:D])
    wk = np.ascontiguousarray(Wqkv[:, D : 2 * D])
    wv = np.ascontiguousarray(Wqkv[:, 2 * D : 3 * D])
    in_maps = []
    for c in range(N_CORES):
        b, half = c // 2, c % 2
        m = {
            "xb": np.ascontiguousarray(x[b]),
            "xq": np.ascontiguousarray(x[b, half * LQ : (half + 1) * LQ, :]),
            "wq": wq,
            "wk": wk,
            "wv": wv,
            "wout": np.ascontiguousarray(Wout),
            "cosk": cos,
            "sink": sin,
            "cosq": np.ascontiguousarray(cos[half * LQ : (half + 1) * LQ]),
            "sinq": np.ascontiguousarray(sin[half * LQ : (half + 1) * LQ]),
        }
        if use_bq:
            m["bq"] = np.ascontiguousarray(bqkv[0:D]).reshape(1, D)
        if use_bk:
            m["bk"] = np.ascontiguousarray(bqkv[D : 2 * D]).reshape(1, D)
        if use_bv:
            m["bv"] = np.ascontiguousarray(bqkv[2 * D : 3 * D]).reshape(1, D)
        if use_bout:
            m["bout"] = np.ascontiguousarray(bout).reshape(1, D)
        if use_qnw:
            m["qnw"] = np.ascontiguousarray(qn_w).reshape(1, HD)
        if use_knw:
            m["knw"] = np.ascontiguousarray(kn_w).reshape(1, HD)
        in_maps.append(m)
    return in_maps


def _flags_for(bqkv, qn_w, kn_w, bout):
    return (
        bool(np.any(bqkv[0:D])),
        bool(np.any(bqkv[D : 2 * D])),
        bool(np.any(bqkv[2 * D : 3 * D])),
        bool(np.any(bout)),
        bool(np.any(qn_w != 1.0)),
        bool(np.any(kn_w != 1.0)),
    )


def _assemble(results):
    out = np.empty((B, L, D), dtype=np.float32)
    for c in range(N_CORES):
        b, half = c // 2, c % 2
        out[b, half * LQ : (half + 1) * LQ, :] = results[c]["out"]
    return out


def kernel(x, Wqkv, bqkv, qn_w, kn_w, Wout, bout, _trace=False):
    x = np.asarray(x, dtype=np.float32)
    Wqkv = np.asarray(Wqkv, dtype=np.float32)
    bqkv = np.asarray(bqkv, dtype=np.float32)
    qn_w = np.asarray(qn_w, dtype=np.float32)
    kn_w = np.asarray(kn_w, dtype=np.float32)
    Wout = np.asarray(Wout, dtype=np.float32)
    bout = np.asarray(bout, dtype=np.float32)

    flags = _flags_for(bqkv, qn_w, kn_w, bout)
    nc = _get_program(flags)
    if not getattr(nc, "_waits_split", False):
        _split_waits(nc)
        nc._waits_split = True
    in_maps = _make_in_maps(x, Wqkv, bqkv, qn_w, kn_w, Wout, bout, flags)
    res = bass_utils.run_bass_kernel_spmd(
        nc, in_maps, core_ids=list(range(N_CORES))
    )
    out = _assemble(res.results)
    if _trace:
        return out, res
    return out
